# revision 2
# baseline (speedup 1.0000x reference)
"""BiLSTM seq2seq with concat-attention + 32k-vocab log_softmax on 8 TRN2 NeuronCores.

Strategy: recurrent phases (encoder BiLSTM, attention decoder) are replicated on
all 8 cores (they are weight-streaming bound, batch=16 is tiny); the output
projection + log_softmax is sharded column-wise over the 32k vocab (4000/core),
with one AllReduce of the per-row sum(exp(logits)) for the normalizer.

All matmuls run in bf16 (fp32 PSUM accumulate); LSTM cell state c and all
elementwise math stay fp32. sigmoid(x) = 0.5*tanh(0.5x)+0.5 so the whole
recurrent loop uses one ACT table set (exp_and_others: tanh+exp).
"""
import sys
import os

sys.path.insert(0, "/opt/trn_rl_repo")

import numpy as np
import ml_dtypes
from contextlib import ExitStack

import concourse.bass as bass
import concourse.tile as tile
from concourse import bacc, mybir
from concourse._compat import with_exitstack
from concourse.masks import make_identity

BF16 = mybir.dt.bfloat16
F32 = mybir.dt.float32
AF = mybir.ActivationFunctionType
ALU = mybir.AluOpType

# Problem constants (hardcoded; kernel.py must be self-contained)
B = 16
E = 512
H = 512
H2 = 1024
G = 2048        # 4*H   encoder gates
GD = 4096       # 4*H2  decoder gates
V = 32000
NCORES = 8
VS = V // NCORES  # 4000


class Cfg:
    def __init__(self, ls=128, lt=128, n_cores=8):
        self.ls = ls            # encoder timesteps
        self.lt = lt            # decoder timesteps
        self.n_cores = n_cores
        self.cb = B * ls        # attention cols (b-major: col = b*ls + l)
        self.rows = B * lt      # output rows (b-major: row = b*lt + t)


def _ceil_div(a, b):
    return (a + b - 1) // b


def _chunks(total, size):
    out = []
    o = 0
    while o < total:
        out.append((o, min(size, total - o)))
        o += size
    return out


@with_exitstack
def _kernel_body(ctx: ExitStack, tc: tile.TileContext, cfg: Cfg, outs, ins):
    nc = tc.nc
    LS, LT, CB, ROWS = cfg.ls, cfg.lt, cfg.cb, cfg.rows

    dram = ctx.enter_context(tc.tile_pool(name="dram", bufs=1, space="DRAM"))
    const = ctx.enter_context(tc.tile_pool(name="const", bufs=1))

    # ---- constants ----
    ident_bf = const.tile([128, 128], BF16)
    make_identity(nc, ident_bf[:])
    ones_bf = const.tile([1, 128], BF16)   # row of ones (bias matmuls, replication)
    nc.vector.memset(ones_bf[:], 1.0)
    onesK_bf = const.tile([128, 1], BF16)  # column of ones (Z sums)
    nc.vector.memset(onesK_bf[:], 1.0)
    ident_f1 = const.tile([1, 1], F32)
    nc.vector.memset(ident_f1[:], 1.0)
    vT = const.tile([128, 8], BF16)
    nc.sync.dma_start(vT[:], ins["vT"][:])
    battnT = const.tile([128, 8], F32)
    nc.sync.dma_start(battnT[:], ins["battnT"][:])

    # decoder state (persistent across phases)
    hT_d = const.tile([128, 8 * 16], BF16)
    c_d = const.tile([16, H2], F32)
    # DRAM intermediates
    preT_dram = dram.tile([8, 128, CB], BF16)        # tanh-arg precompute (d-tiles)
    encrow2_dram = dram.tile([8, LS, B * 128], BF16)  # enc_out as [dt][l,(b,dsub)]
    xwd_dram = dram.tile([ROWS, GD], BF16)           # x@Wih_d[:E] + b_d, rows (t,b)
    hsT_dram = dram.tile([8, 128, ROWS], BF16)       # decoder hs^T, cols (b,t)
    logits_dram = dram.tile([ROWS, VS], BF16)        # relu'd logits shard
    cc_in = dram.tile([128, 16], F32)                # collective bounce
    cc_out = dram.tile([128, 16], F32)

    # =====================================================================
    # Phases A+B share encT; their pools free before the decoder
    # =====================================================================
    phAB = ctx.enter_context(ExitStack())
    pab = phAB.enter_context(tc.tile_pool(name="phAB", bufs=1))
    # enc_out^T [H2, CB] bf16: d-tile j at cols [j*CB,(j+1)*CB); d<512 fwd, >=512 bwd
    encT = pab.tile([128, 8 * CB], BF16)
    hT_f = pab.tile([128, 4 * 16], BF16)   # (h_f)^T, k-tile j at cols [16j,16j+16)
    hT_b = pab.tile([128, 4 * 16], BF16)
    c_f = pab.tile([16, H], F32)
    c_b = pab.tile([16, H], F32)
    nc.vector.memset(hT_f[:], 0.0)
    nc.vector.memset(hT_b[:], 0.0)
    nc.vector.memset(c_f[:], 0.0)
    nc.vector.memset(c_b[:], 0.0)

    # Phase A: encoder BiLSTM (fwd + bwd interleaved)
    with ExitStack() as phA:
        pa = phA.enter_context(tc.tile_pool(name="phA", bufs=1))
        xsT = pa.tile([128, 4 * CB], BF16)     # x^T k-tiles, cols (b,l)
        Wenc_f = pa.tile([128, 8 * G], BF16)   # k-tiles: 0-3 Wih, 4-7 Whh
        Wenc_b = pa.tile([128, 8 * G], BF16)
        nc.sync.dma_start(xsT[:], ins["xsT_t"][:])
        nc.sync.dma_start(Wenc_f[:], ins["Wenc_f_t"][:])
        nc.sync.dma_start(Wenc_b[:], ins["Wenc_b_t"][:])

        gl = phA.enter_context(tc.tile_pool(name="eg", bufs=1))   # gate sbuf tiles
        eg_ps = phA.enter_context(tc.tile_pool(name="eg_ps", bufs=3, space="PSUM"))
        tp_ps = phA.enter_context(tc.tile_pool(name="tp_psA", bufs=2, space="PSUM"))

        def enc_step(t_dir, hT, c_st, W, dvi):
            # gates = [x_t; h]^T.T @ W  -> 4 chunks [16,512] (i,f,g,o)
            gates = []
            for cch in range(4):
                ps = eg_ps.tile([16, 512], F32, tag="eg_ps")
                for kt in range(4):  # x part; lhsT = xsT cols {b*LS + t}
                    xsl = bass.AP(
                        tensor=xsT.tensor,
                        offset=xsT.offset + kt * CB + t_dir,
                        ap=[xsT.ap[0], [LS, 16]],
                    )
                    nc.tensor.matmul(ps[:], lhsT=xsl, rhs=W[:, kt * G + cch * 512:kt * G + cch * 512 + 512],
                                     start=(kt == 0), stop=False)
                for kt in range(4):  # h part
                    nc.tensor.matmul(ps[:], lhsT=hT[:, kt * 16:kt * 16 + 16],
                                     rhs=W[:, (4 + kt) * G + cch * 512:(4 + kt) * G + cch * 512 + 512],
                                     start=False, stop=(kt == 3))
                gates.append(ps)
            # cell: sigmoid via tanh; c fp32
            ti = gl.tile([16, 512], F32, tag=f"ti{dvi}")
            tf = gl.tile([16, 512], F32, tag=f"tf{dvi}")
            tg = gl.tile([16, 512], F32, tag=f"tg{dvi}")
            to = gl.tile([16, 512], F32, tag=f"to{dvi}")
            nc.scalar.activation(ti[:], gates[0][:], AF.Tanh, scale=0.5)
            nc.scalar.activation(tf[:], gates[1][:], AF.Tanh, scale=0.5)
            nc.scalar.activation(tg[:], gates[2][:], AF.Tanh)
            nc.scalar.activation(to[:], gates[3][:], AF.Tanh, scale=0.5)
            nc.vector.tensor_scalar(out=ti[:], in0=ti[:], scalar1=0.5, scalar2=0.5, op0=ALU.mult, op1=ALU.add)
            nc.vector.tensor_scalar(out=tf[:], in0=tf[:], scalar1=0.5, scalar2=0.5, op0=ALU.mult, op1=ALU.add)
            nc.vector.tensor_scalar(out=to[:], in0=to[:], scalar1=0.5, scalar2=0.5, op0=ALU.mult, op1=ALU.add)
            nc.vector.tensor_tensor(out=tf[:], in0=tf[:], in1=c_st[:], op=ALU.mult)
            nc.vector.tensor_tensor(out=tg[:], in0=ti[:], in1=tg[:], op=ALU.mult)
            nc.vector.tensor_tensor(out=c_st[:], in0=tf[:], in1=tg[:], op=ALU.add)
            nc.scalar.activation(ti[:], c_st[:], AF.Tanh)
            h_bf = gl.tile([16, H], BF16, tag=f"hbf{dvi}")
            nc.vector.tensor_tensor(out=h_bf[:], in0=to[:], in1=ti[:], op=ALU.mult)
            # transpose h -> hT (4 blocks) and scatter into encT cols {b*LS + t}
            for j in range(4):
                pst = tp_ps.tile([128, 16], BF16, tag="tp")
                nc.tensor.transpose(pst[:], h_bf[:, j * 128:(j + 1) * 128], ident_bf[0:16, 0:16])
                nc.vector.tensor_copy(hT[:, j * 16:j * 16 + 16], pst[:])
                dtile = j if dvi == 0 else 4 + j
                dst = bass.AP(tensor=encT.tensor, offset=encT.offset + dtile * CB + t_dir,
                              ap=[encT.ap[0], [LS, 16]])
                nc.vector.tensor_copy(dst, pst[:])

        for t in range(LS):
            enc_step(t, hT_f, c_f, Wenc_f, 0)
            enc_step(LS - 1 - t, hT_b, c_b, Wenc_b, 1)

        # init decoder state from encoder finals: h = [h_f; h_b], c = [c_f; c_b]
        nc.vector.tensor_copy(hT_d[:, 0:64], hT_f[:, :])
        nc.vector.tensor_copy(hT_d[:, 64:128], hT_b[:, :])
        nc.vector.tensor_copy(c_d[:, 0:H], c_f[:])
        nc.vector.tensor_copy(c_d[:, H:H2], c_b[:])

    # =====================================================================
    # Phase B: preT = (enc_out @ W_bot + b_attn)^T  -> DRAM (bf16)
    #          encrow2[dt] = enc_out[b,l,dsub]^T blocks -> DRAM
    # =====================================================================
    with ExitStack() as phB:
        pb = phB.enter_context(tc.tile_pool(name="phB", bufs=1))
        Wbot = pb.tile([128, 8 * H2], BF16)
        nc.sync.dma_start(Wbot[:], ins["Wbot_t"][:])
        stg = phB.enter_context(tc.tile_pool(name="phB_stg", bufs=3))
        pb_ps = phB.enter_context(tc.tile_pool(name="phB_ps", bufs=3, space="PSUM"))
        for m in range(8):  # out d-tile
            for (co, cn) in _chunks(CB, 512):
                ps = pb_ps.tile([128, 512], F32, tag="pre_ps")
                for kt in range(8):
                    nc.tensor.matmul(ps[0:128, 0:cn],
                                     lhsT=Wbot[:, kt * H2 + m * 128:kt * H2 + m * 128 + 128],
                                     rhs=encT[:, kt * CB + co:kt * CB + co + cn],
                                     start=(kt == 0), stop=(kt == 7))
                sb = stg.tile([128, 512], BF16, tag="pre_sb")
                nc.scalar.activation(sb[0:128, 0:cn], ps[0:128, 0:cn], AF.Identity,
                                     bias=battnT[:, m:m + 1])
                nc.sync.dma_start(preT_dram[m, :, co:co + cn], sb[0:128, 0:cn])
        # encrow2: transpose encT blocks [d x l] -> [l x d] per (b, dt)
        for dt in range(8):
            for b in range(B):
                pst = pb_ps.tile([128, 128], BF16, tag="er_ps")
                nc.tensor.transpose(pst[0:LS, 0:128],
                                    encT[:, dt * CB + b * LS:dt * CB + b * LS + LS],
                                    ident_bf[:, :])
                sb = stg.tile([128, 128], BF16, tag="er_sb")
                nc.vector.tensor_copy(sb[0:LS, :], pst[0:LS, :])
                nc.sync.dma_start(encrow2_dram[dt, :, b * 128:(b + 1) * 128], sb[0:LS, :])

    phAB.close()

    # =====================================================================
    # Phase B2: xwd = dec_x @ Wih_d[:E] + b_d  -> DRAM (rows (t,b))
    # =====================================================================
    with ExitStack() as phB2:
        pb2 = phB2.enter_context(tc.tile_pool(name="phB2", bufs=1))
        decT = pb2.tile([128, 4 * ROWS], BF16)
        Wdx = pb2.tile([128, 4 * GD], BF16)
        bd = pb2.tile([1, GD], BF16)
        nc.sync.dma_start(decT[:], ins["decT_t"][:])
        nc.sync.dma_start(Wdx[:], ins["Wdx_t"][:])
        nc.sync.dma_start(bd[:], ins["bd"][:])
        stg = phB2.enter_context(tc.tile_pool(name="phB2_stg", bufs=3))
        pb2_ps = phB2.enter_context(tc.tile_pool(name="phB2_ps", bufs=3, space="PSUM"))
        for m in range(_ceil_div(ROWS, 128)):
            mo = m * 128
            mn = min(128, ROWS - mo)
            for cch in range(8):
                ps = pb2_ps.tile([128, 512], F32, tag="xw_ps")
                for kt in range(4):
                    nc.tensor.matmul(ps[0:mn, :],
                                     lhsT=decT[:, kt * ROWS + mo:kt * ROWS + mo + mn],
                                     rhs=Wdx[:, kt * GD + cch * 512:kt * GD + cch * 512 + 512],
                                     start=(kt == 0), stop=False)
                nc.tensor.matmul(ps[0:mn, :], lhsT=ones_bf[:, 0:mn],
                                 rhs=bd[:, cch * 512:cch * 512 + 512],
                                 start=False, stop=True)
                sb = stg.tile([128, 512], BF16, tag="xw_sb")
                nc.vector.tensor_copy(sb[0:mn, :], ps[0:mn, :])
                nc.sync.dma_start(xwd_dram[mo:mo + mn, cch * 512:cch * 512 + 512], sb[0:mn, :])

    # =====================================================================
    # Phase C: attention decoder
    # =====================================================================
    with ExitStack() as phC:
        pc = phC.enter_context(tc.tile_pool(name="phC", bufs=1))
        Wd = pc.tile([128, 16 * GD], BF16)   # k 0-7: cvec part, 8-15: h part
        Wtop = pc.tile([128, 8 * H2], BF16)
        nc.sync.dma_start(Wd[:], ins["Wd_t"][:])
        nc.sync.dma_start(Wtop[:], ins["Wtop_t"][:])

        cvT = pc.tile([128, 8 * 16], BF16)
        hWT = pc.tile([128, 8 * 16], BF16)   # (h @ Wtop)^T, d-tile j at cols 16j
        hw_sb = pc.tile([16, H2], BF16)
        e_sb = pc.tile([1, CB], F32)
        wT = pc.tile([128, 16], F32)         # exp(e)^T [l, b]
        wT_bf = pc.tile([128, 16], BF16)
        rZ = pc.tile([1, 16], F32)
        rZ_bf = pc.tile([1, 16], BF16)
        wn_bf = pc.tile([128, 16], BF16)     # softmax weights^T bf16
        tg4 = pc.tile([16, 4 * H2], F32)     # gate tiles i,f,g,o
        h_bf = pc.tile([16, H2], BF16)

        strm = phC.enter_context(tc.tile_pool(name="strm", bufs=8))   # preT halves
        er_pool = phC.enter_context(tc.tile_pool(name="er", bufs=2))  # encrow2 tiles
        xw_pool = phC.enter_context(tc.tile_pool(name="xw", bufs=2))
        garg_pool = phC.enter_context(tc.tile_pool(name="garg", bufs=2))

        e_ps_pool = phC.enter_context(tc.tile_pool(name="e_ps", bufs=2, space="PSUM"))
        tp_pool = phC.enter_context(tc.tile_pool(name="tp_ps", bufs=4, space="PSUM"))
        g_ps_pool = phC.enter_context(tc.tile_pool(name="g_ps", bufs=2, space="PSUM"))

        HCB = CB // 2 if CB >= 1024 else CB   # column half size for attention tiles
        n_half = _ceil_div(CB, HCB)

        for t in range(LT):
            # ---- hW = h @ Wtop  [16, H2] -> hw_sb (bf16) -> hWT (transposed)
            for cch in range(2):
                ps = g_ps_pool.tile([16, 512], F32, tag="g16")
                for kt in range(8):
                    nc.tensor.matmul(ps[:], lhsT=hT_d[:, kt * 16:kt * 16 + 16],
                                     rhs=Wtop[:, kt * H2 + cch * 512:kt * H2 + cch * 512 + 512],
                                     start=(kt == 0), stop=(kt == 7))
                nc.scalar.activation(hw_sb[:, cch * 512:cch * 512 + 512], ps[:], AF.Copy)
            for j in range(8):
                pst = tp_pool.tile([128, 16], BF16, tag="tp")
                nc.tensor.transpose(pst[:], hw_sb[:, j * 128:(j + 1) * 128], ident_bf[0:16, 0:16])
                nc.vector.tensor_copy(hWT[:, j * 16:j * 16 + 16], pst[:])

            # ---- attention: arg = preT + hWT (bcast over l); tanh; e = v.T @ tanh
            e_chunks = _chunks(CB, 512)
            e_ps_tiles = {}
            for half in range(n_half):
                ho = half * HCB
                tanh_tiles = []
                for dt2 in range(8):
                    pre = strm.tile([128, HCB], BF16, tag="pre")
                    nc.sync.dma_start(pre[:], preT_dram[dt2, :, ho:ho + HCB])
                    # in-place: pre += hWT[:,dt2-block] broadcast over l
                    nb = HCB // LS
                    pre3 = pre.rearrange("p (b l) -> p b l", l=LS)
                    hb = bass.AP(tensor=hWT.tensor, offset=hWT.offset + dt2 * 16 + (ho // LS),
                                 ap=[hWT.ap[0], [1, nb], [0, LS]])
                    nc.vector.tensor_tensor(out=pre3, in0=pre3, in1=hb, op=ALU.add)
                    nc.scalar.activation(pre[:], pre[:], AF.Tanh)
                    tanh_tiles.append(pre)
                for (co, cn) in _chunks(HCB, 512):
                    eps = e_ps_pool.tile([1, 512], F32, tag="e_ps")
                    e_ps_tiles[ho + co] = (eps, cn)
                    for dt2 in range(8):
                        nc.tensor.matmul(eps[0:1, 0:cn], lhsT=vT[:, dt2:dt2 + 1],
                                         rhs=tanh_tiles[dt2][:, co:co + cn],
                                         start=(dt2 == 0), stop=(dt2 == 7))
                for (co, cn) in _chunks(HCB, 512):
                    eps, _ = e_ps_tiles[ho + co]
                    nc.scalar.activation(e_sb[0:1, ho + co:ho + co + cn], eps[0:1, 0:cn], AF.Copy)

            # ---- softmax pieces: eT (transposes), exp, Z, 1/Z, wn
            eT_ps = tp_pool.tile([128, 16], F32, tag="tp")
            for b in range(B):
                nc.tensor.transpose(eT_ps[0:LS, b:b + 1], e_sb[0:1, b * LS:b * LS + LS],
                                    ident_f1[:])
            nc.scalar.activation(wT[0:LS, :], eT_ps[0:LS, :], AF.Exp)
            nc.vector.tensor_copy(wT_bf[0:LS, :], wT[0:LS, :])
            z_ps = tp_pool.tile([128, 16], F32, tag="tp")
            nc.tensor.matmul(z_ps[0:1, :], lhsT=onesK_bf[0:LS, :], rhs=wT_bf[0:LS, :],
                             start=True, stop=True)
            nc.vector.reciprocal(rZ[:], z_ps[0:1, :])
            nc.vector.tensor_copy(rZ_bf[:], rZ[:])
            rep_ps = tp_pool.tile([128, 16], F32, tag="tp")
            nc.tensor.matmul(rep_ps[0:LS, :], lhsT=ones_bf[:, 0:LS], rhs=rZ_bf[:],
                             start=True, stop=True)
            nc.vector.tensor_tensor(out=wn_bf[0:LS, :], in0=wT[0:LS, :], in1=rep_ps[0:LS, :],
                                    op=ALU.mult)

            # ---- cvec^T: per d-tile, 16 per-b matmuls (lhsT = encrow2 block)
            for dt2 in range(8):
                er = er_pool.tile([LS, B * 128], BF16, tag="er")
                nc.sync.dma_start(er[:], encrow2_dram[dt2, 0:LS, :])
                cps = tp_pool.tile([128, 16], F32, tag="tp")
                for b in range(B):
                    nc.tensor.matmul(cps[:, b:b + 1], lhsT=er[:, b * 128:(b + 1) * 128],
                                     rhs=wn_bf[0:LS, b:b + 1], start=True, stop=True)
                nc.vector.tensor_copy(cvT[:, dt2 * 16:dt2 * 16 + 16], cps[:])

            # ---- gates = [cvec; h] @ Wd + xwd[t]
            for cch in range(8):
                xw = xw_pool.tile([16, 512], BF16, tag="xw")
                nc.sync.dma_start(xw[:], xwd_dram[t * 16:(t + 1) * 16, cch * 512:cch * 512 + 512])
                ps = g_ps_pool.tile([16, 512], F32, tag="g16")
                for kt in range(8):
                    nc.tensor.matmul(ps[:], lhsT=cvT[:, kt * 16:kt * 16 + 16],
                                     rhs=Wd[:, kt * GD + cch * 512:kt * GD + cch * 512 + 512],
                                     start=(kt == 0), stop=False)
                for kt in range(8):
                    nc.tensor.matmul(ps[:], lhsT=hT_d[:, kt * 16:kt * 16 + 16],
                                     rhs=Wd[:, (8 + kt) * GD + cch * 512:(8 + kt) * GD + cch * 512 + 512],
                                     start=False, stop=(kt == 7))
                garg = garg_pool.tile([16, 512], F32, tag="garg")
                nc.vector.tensor_tensor(out=garg[:], in0=ps[:], in1=xw[:], op=ALU.add)
                # gate nonlinearity straight into tg4: chunks 0,1=i 2,3=f 4,5=g 6,7=o
                gate = cch // 2
                half5 = (cch % 2) * 512
                scale = 1.0 if gate == 2 else 0.5
                nc.scalar.activation(tg4[:, gate * H2 + half5:gate * H2 + half5 + 512],
                                     garg[:], AF.Tanh, scale=scale)

            # ---- cell (slices of tg4: i,f,g,o at cols 0,H2,2H2,3H2)
            ti = tg4[:, 0:H2]
            tf = tg4[:, H2:2 * H2]
            tg = tg4[:, 2 * H2:3 * H2]
            to = tg4[:, 3 * H2:4 * H2]
            nc.vector.tensor_scalar(out=ti, in0=ti, scalar1=0.5, scalar2=0.5, op0=ALU.mult, op1=ALU.add)
            nc.vector.tensor_scalar(out=tf, in0=tf, scalar1=0.5, scalar2=0.5, op0=ALU.mult, op1=ALU.add)
            nc.vector.tensor_scalar(out=to, in0=to, scalar1=0.5, scalar2=0.5, op0=ALU.mult, op1=ALU.add)
            nc.vector.tensor_tensor(out=tf, in0=tf, in1=c_d[:], op=ALU.mult)
            nc.vector.tensor_tensor(out=tg, in0=ti, in1=tg, op=ALU.mult)
            nc.vector.tensor_tensor(out=c_d[:], in0=tf, in1=tg, op=ALU.add)
            nc.scalar.activation(ti, c_d[:], AF.Tanh)
            nc.vector.tensor_tensor(out=h_bf[:], in0=to, in1=ti, op=ALU.mult)

            # ---- h -> hT_d (8 transposes); also ship columns to hsT_dram
            for j in range(8):
                pst = tp_pool.tile([128, 16], BF16, tag="tp")
                nc.tensor.transpose(pst[:], h_bf[:, j * 128:(j + 1) * 128], ident_bf[0:16, 0:16])
                nc.vector.tensor_copy(hT_d[:, j * 16:j * 16 + 16], pst[:])
                dst = bass.AP(tensor=hsT_dram.tensor,
                              offset=hsT_dram.offset + j * (128 * ROWS) + t,
                              ap=[[ROWS, 128], [LT, 16]])
                nc.sync.dma_start(dst, hT_d[:, j * 16:j * 16 + 16])

    # =====================================================================
    # Phase D: logits shard = relu(hs @ W_out[:,shard] + b_out[shard]);
    # sumexp -> AllReduce -> out = logits - log(Z)
    # =====================================================================
    with ExitStack() as phD:
        pd = phD.enter_context(tc.tile_pool(name="phD", bufs=1))
        WoT = pd.tile([128, 8 * VS], BF16)
        bo = pd.tile([1, VS], BF16)
        nc.sync.dma_start(WoT[:], ins["WoT_t"][:])
        nc.sync.dma_start(bo[:], ins["bo"][:])
        n_mt = _ceil_div(ROWS, 128)
        sumZ = pd.tile([128, 16], F32)
        nlogZ = pd.tile([128, 16], F32)
        scr = pd.tile([128, VS], BF16)
        nc.vector.memset(sumZ[:], 1.0)

        pdm = phD.enter_context(tc.tile_pool(name="phD_m", bufs=2))
        pd_ps = phD.enter_context(tc.tile_pool(name="phD_ps", bufs=3, space="PSUM"))

        vchunks = _chunks(VS, 500)
        for m in range(n_mt):
            mo = m * 128
            mn = min(128, ROWS - mo)
            hsm = pdm.tile([128, 8 * mn], BF16, tag="hsm")
            hs_src = bass.AP(tensor=hsT_dram.tensor, offset=hsT_dram.offset + mo,
                             ap=[[ROWS, 128], [128 * ROWS, 8], [1, mn]])
            nc.sync.dma_start(hsm[0:128, 0:8 * mn].rearrange("p (k r) -> p k r", k=8), hs_src)
            lr = pdm.tile([128, VS], BF16, tag="lr")
            for (co, cn) in vchunks:
                ps = pd_ps.tile([128, 512], F32, tag="lg_ps")
                for kt in range(8):
                    nc.tensor.matmul(ps[0:mn, 0:cn], lhsT=hsm[:, kt * mn:kt * mn + mn],
                                     rhs=WoT[:, kt * VS + co:kt * VS + co + cn],
                                     start=(kt == 0), stop=False)
                nc.tensor.matmul(ps[0:mn, 0:cn], lhsT=ones_bf[:, 0:mn], rhs=bo[:, co:co + cn],
                                 start=False, stop=True)
                nc.scalar.activation(lr[0:mn, co:co + cn], ps[0:mn, 0:cn], AF.Relu)
            nc.sync.dma_start(logits_dram[mo:mo + mn, :], lr[0:mn, :])
            nc.scalar.activation(scr[0:mn, :], lr[0:mn, :], AF.Exp,
                                 accum_out=sumZ[0:mn, m:m + 1])

        # AllReduce of per-row sum(exp) across vocab shards
        nc.sync.dma_start(cc_in[:, :], sumZ[:, :])
        if cfg.n_cores > 1:
            nc.gpsimd.collective_compute(
                "AllReduce", ALU.add,
                replica_groups=[list(range(cfg.n_cores))],
                ins=[cc_in.opt()], outs=[cc_out.opt()],
            )
            nc.sync.dma_start(sumZ[:, :], cc_out[:, :])
        nc.scalar.activation(nlogZ[:, :], sumZ[:, :], AF.Ln)
        nc.vector.tensor_scalar(out=nlogZ[:, :], in0=nlogZ[:, :], scalar1=-1.0, scalar2=None,
                                op0=ALU.mult)

        for m in range(n_mt):
            mo = m * 128
            mn = min(128, ROWS - mo)
            lr = pdm.tile([128, VS], BF16, tag="lr2")
            nc.sync.dma_start(lr[0:mn, :], logits_dram[mo:mo + mn, :])
            of = pdm.tile([128, VS], F32, tag="of")
            nc.scalar.activation(of[0:mn, :], lr[0:mn, :], AF.Identity,
                                 bias=nlogZ[0:mn, m:m + 1])
            nc.sync.dma_start(outs["out_shard"][mo:mo + mn, :], of[0:mn, :])


# ---------------------------------------------------------------------------
# host side
# ---------------------------------------------------------------------------

def _tile_k(mat: np.ndarray) -> np.ndarray:
    """[K, N] -> [128, (K//128)*N] with k-tile kt at cols [kt*N,(kt+1)*N)."""
    k, n = mat.shape
    assert k % 128 == 0
    return np.ascontiguousarray(mat.reshape(k // 128, 128, n).transpose(1, 0, 2).reshape(128, -1))


def _bf(x):
    return np.asarray(x, dtype=np.float32).astype(ml_dtypes.bfloat16)


_PROG_CACHE = {}


def _build_program(cfg: Cfg):
    key = (cfg.ls, cfg.lt, cfg.n_cores)
    if key in _PROG_CACHE:
        return _PROG_CACHE[key]
    nc = bacc.Bacc("TRN2", target_bir_lowering=False, debug=False,
                   enable_asserts=False, num_devices=cfg.n_cores)
    ins = {}

    def inp(name, shape, dt):
        ins[name] = nc.dram_tensor(name, list(shape), dt, kind="ExternalInput").ap()

    inp("xsT_t", (128, 4 * cfg.cb), BF16)
    inp("decT_t", (128, 4 * cfg.rows), BF16)
    inp("Wenc_f_t", (128, 8 * G), BF16)
    inp("Wenc_b_t", (128, 8 * G), BF16)
    inp("Wtop_t", (128, 8 * H2), BF16)
    inp("Wbot_t", (128, 8 * H2), BF16)
    inp("battnT", (128, 8), F32)
    inp("vT", (128, 8), BF16)
    inp("Wdx_t", (128, 4 * GD), BF16)
    inp("bd", (1, GD), BF16)
    inp("Wd_t", (128, 16 * GD), BF16)
    inp("WoT_t", (128, 8 * VS), BF16)
    inp("bo", (1, VS), BF16)
    outs = {"out_shard": nc.dram_tensor("out_shard", [cfg.rows, VS], F32,
                                        kind="ExternalOutput").ap()}
    with tile.TileContext(nc) as tc:
        _kernel_body(tc, cfg, outs, ins)
    nc.compile()
    _PROG_CACHE[key] = nc
    return nc


def prep_in_maps(inputs: dict, cfg: Cfg):
    f32 = lambda k: np.asarray(inputs[k], dtype=np.float32)
    inp_idx = np.asarray(inputs["inp"]).astype(np.int64)[:, :cfg.ls]
    tar_idx = np.asarray(inputs["tar"]).astype(np.int64)[:, :cfg.lt]
    enc_emb = f32("enc_emb")
    dec_emb = f32("dec_emb")

    xs = enc_emb[inp_idx]                       # [B, LS, E]
    xsT = xs.reshape(cfg.cb, E).T               # [E, CB] cols (b,l)
    dec_x = dec_emb[tar_idx].transpose(1, 0, 2).reshape(cfg.rows, E)  # rows (t,b)
    decT = dec_x.T

    Wenc_f = np.concatenate([f32("Wih_f"), f32("Whh_f")], 0)
    Wenc_b = np.concatenate([f32("Wih_b"), f32("Whh_b")], 0)
    W_attn = f32("W_attn")
    Wih_d = f32("Wih_d")
    Whh_d = f32("Whh_d")
    Wd = np.concatenate([Wih_d[E:E + H2], Whh_d], 0)

    base = {
        "xsT_t": _bf(_tile_k(xsT)),
        "decT_t": _bf(_tile_k(decT)),
        "Wenc_f_t": _bf(_tile_k(Wenc_f)),
        "Wenc_b_t": _bf(_tile_k(Wenc_b)),
        "Wtop_t": _bf(_tile_k(W_attn[:H2])),
        "Wbot_t": _bf(_tile_k(W_attn[H2:])),
        "battnT": np.ascontiguousarray(f32("b_attn").reshape(8, 128).T),
        "vT": _bf(f32("v_attn").reshape(8, 128).T),
        "Wdx_t": _bf(_tile_k(Wih_d[:E])),
        "bd": _bf(f32("b_d").reshape(1, GD)),
        "Wd_t": _bf(_tile_k(Wd)),
    }
    W_out = f32("W_out")
    b_out = f32("b_out")
    in_maps = []
    for c in range(cfg.n_cores):
        m = dict(base)
        m["WoT_t"] = _bf(_tile_k(W_out[:, c * VS:(c + 1) * VS]))
        m["bo"] = _bf(b_out[c * VS:(c + 1) * VS].reshape(1, VS))
        in_maps.append(m)
    return in_maps


LAST_EXEC_NS = None


def kernel(**inputs) -> np.ndarray:
    global LAST_EXEC_NS
    cfg = Cfg(ls=128, lt=128, n_cores=NCORES)
    nc = _build_program(cfg)
    in_maps = prep_in_maps(inputs, cfg)
    from concourse.bass_utils import run_bass_kernel_spmd
    trace = os.environ.get("KERNEL_TRACE") == "1"
    res = run_bass_kernel_spmd(nc, in_maps, core_ids=list(range(cfg.n_cores)),
                               trace=trace,
                               tmpdir=os.environ.get("KERNEL_TRACE_DIR"))
    LAST_EXEC_NS = res.exec_time_ns
    shards = [res.results[i]["out_shard"].reshape(B, cfg.lt, VS)
              for i in range(cfg.n_cores)]
    return np.concatenate(shards, axis=2).astype(np.float32)



# revision 3
# speedup vs baseline: 1.0212x; 1.0212x over previous
"""BiLSTM seq2seq + concat-attention + 32k-vocab log_softmax on 8 TRN2 cores.

v2 strategy (vs v1 replicate-recurrence / vocab-shard):
  - Data-parallel over batch: each core owns 2 of the 16 sequences and runs the
    FULL model for them, including the full-vocab output projection and
    log_softmax. No collectives at all; host concatenates batch shards.
  - All recurrent weight-streaming matmuls (encoder Whh, decoder Wd/Wtop, the
    attention e-reduction, and the output projection) run in fp8e4m3 with the
    DoubleRow perf mode (2 k-tiles per pass -> 2x PE throughput). Weights are
    pre-scaled x16 on the host to clear the fp8 denormal region; the 1/16 is
    folded into activation scales.
  - x @ Wih (+bias) terms for both encoder dirs and the decoder are hoisted out
    of the recurrences into one batched precompute (P0), streamed back per step
    as [2, 4H] rows and injected into PSUM via an identity matmul.
  - The whole decoder working set (Wd, Wtop, preT, enc rows, hs^T) lives in
    SBUF; the only DMA in the recurrent loops is the tiny per-step xw row.
"""
import os
import sys

sys.path.insert(0, "/opt/trn_rl_repo")

import numpy as np
import ml_dtypes
from contextlib import ExitStack

import concourse.bass as bass
import concourse.tile as tile
from concourse import bacc, mybir
from concourse._compat import with_exitstack
from concourse.masks import make_identity

BF16 = mybir.dt.bfloat16
F32 = mybir.dt.float32
FP8 = mybir.dt.float8e4
AF = mybir.ActivationFunctionType
ALU = mybir.AluOpType
DR = mybir.MatmulPerfMode.DoubleRow

# problem constants (kernel.py must be self-contained)
B, E, H, H2 = 16, 512, 512, 1024
G = 4 * H           # 2048 encoder gate width
GD = 4 * H2         # 4096 decoder gate width
V = 32000
LS = LT = 128
NCORES = 8
BL = B // NCORES    # 2 sequences per core
RS = BL * LT        # 256 decoder rows, (t, b)-major
CBC = BL * LS       # 256 attention columns per core, (b, l)-major
WS = 16.0           # fp8 weight pre-scale
VCH = 500           # phase-D vocab chunk (one PSUM bank)
NVCH = V // VCH     # 64


def ap3(t, off, s1, c1, s2, c2):
    """3-dim AP view of tile t: [partition, [s1,c1], [s2,c2]] at element offset."""
    return bass.AP(tensor=t.tensor, offset=t.offset + off,
                   ap=[t.ap[0], [s1, c1], [s2, c2]])


@with_exitstack
def _kernel_body(ctx: ExitStack, tc: tile.TileContext, outs, ins):
    nc = tc.nc

    dram = ctx.enter_context(tc.tile_pool(name="dram", bufs=1, space="DRAM"))
    const = ctx.enter_context(tc.tile_pool(name="const", bufs=1))

    ident_bf = const.tile([128, 128], BF16)
    make_identity(nc, ident_bf[:])
    ident_f1 = const.tile([1, 1], F32)
    nc.vector.memset(ident_f1[:], 1.0)
    ones_r = const.tile([1, 128], BF16)     # row of ones (bias matmuls / bcast)
    nc.vector.memset(ones_r[:], 1.0)
    onesK = const.tile([128, 1], BF16)      # column of ones (Z sums)
    nc.vector.memset(onesK[:], 1.0)
    vT_q = const.tile([128, 8 * 32], FP8)
    nc.sync.dma_start(vT_q[:], ins["vT_q"][:])
    battnT = const.tile([128, 8], F32)
    nc.sync.dma_start(battnT[:], ins["battnT"][:])

    # DRAM intermediates: hoisted x-projections
    xwf_dram = dram.tile([CBC, G], BF16)    # rows (l, b): row = 2*l + b
    xwb_dram = dram.tile([CBC, G], BF16)
    xwd_dram = dram.tile([RS, GD], BF16)    # rows (t, b): row = 2*t + b

    # decoder-persistent SBUF (allocated up front; loaded while P0/enc run).
    # decw closes before phase D to make room for the logits stash.
    decw_stack = ctx.enter_context(ExitStack())
    dec = decw_stack.enter_context(tc.tile_pool(name="dec", bufs=1))
    Wd_q = dec.tile([128, 16 * GD], FP8)    # kt 0-7 cv-part, 8-15 h-part (x16)
    nc.sync.dma_start(Wd_q[:], ins["Wd_q"][:])
    Wtop_q = dec.tile([128, 8 * H2], FP8)   # attention h-projection (x16)
    nc.sync.dma_start(Wtop_q[:], ins["Wtop_q"][:])
    preT_sb = dec.tile([128, 8 * CBC], BF16)   # (enc_out @ Wbot + b)^T, dt-major
    encrow = dec.tile([128, BL * H2], BF16)    # enc_out rows: [l, (b, d)]
    hsT_q = const.tile([128, 8 * RS], FP8)     # decoder hs^T k-tiles for phase D
    hT_d = const.tile([128, 8 * 32], FP8)      # decoder h^T, cols (kt, b) b-padded to 32
    c_d = const.tile([BL, H2], F32)
    nc.vector.memset(hT_d[:], 0.0)

    # =====================================================================
    # P0: xwf/xwb/xwd = x @ Wih (+bias), x16, to DRAM
    # =====================================================================
    with ExitStack() as ph0:
        p0 = ph0.enter_context(tc.tile_pool(name="p0", bufs=1))
        xsT = p0.tile([128, 4 * CBC], BF16)
        decT = p0.tile([128, 4 * RS], BF16)
        Wf = p0.tile([128, 4 * G], BF16)
        Wb = p0.tile([128, 4 * G], BF16)
        Wdx = p0.tile([128, 4 * GD], BF16)
        bfb = p0.tile([1, 2 * G], BF16)
        bdr = p0.tile([1, GD], BF16)
        nc.sync.dma_start(xsT[:], ins["xsT_t"][:])
        nc.sync.dma_start(decT[:], ins["decT_t"][:])
        nc.sync.dma_start(Wf[:], ins["Wihf_t"][:])
        nc.sync.dma_start(Wb[:], ins["Wihb_t"][:])
        nc.sync.dma_start(Wdx[:], ins["Wdx_t"][:])
        nc.sync.dma_start(bfb[:], ins["bfb"][:])
        nc.sync.dma_start(bdr[:], ins["bd"][:])
        stg = ph0.enter_context(tc.tile_pool(name="p0s", bufs=3))
        pp = ph0.enter_context(tc.tile_pool(name="p0p", bufs=3, space="PSUM"))

        for W, bi, dst in ((Wf, 0, xwf_dram), (Wb, 1, xwb_dram)):
            for mt in range(BL):       # batch-elem m-tile (xsT cols are b-major)
                for c in range(4):
                    ps = pp.tile([128, 512], F32, tag="ps")
                    for kt in range(4):
                        nc.tensor.matmul(
                            ps[:], lhsT=xsT[:, kt * CBC + mt * 128:kt * CBC + mt * 128 + 128],
                            rhs=W[:, kt * G + c * 512:kt * G + c * 512 + 512],
                            start=(kt == 0), stop=False)
                    nc.tensor.matmul(ps[:], lhsT=ones_r[:, 0:128],
                                     rhs=bfb[0:1, bi * G + c * 512:bi * G + c * 512 + 512],
                                     start=False, stop=True)
                    sb = stg.tile([128, 512], BF16, tag="sb")
                    nc.scalar.activation(sb[:], ps[:], AF.Identity, scale=WS)
                    dst_ap = bass.AP(tensor=dst.tensor,
                                     offset=dst.offset + mt * G + c * 512,
                                     ap=[[BL * G, 128], [1, 512]])
                    nc.sync.dma_start(dst_ap, sb[:])
        for mt in range(2):            # row m-tiles of (t, b)
            for c in range(8):
                ps = pp.tile([128, 512], F32, tag="ps")
                for kt in range(4):
                    nc.tensor.matmul(
                        ps[:], lhsT=decT[:, kt * RS + mt * 128:kt * RS + mt * 128 + 128],
                        rhs=Wdx[:, kt * GD + c * 512:kt * GD + c * 512 + 512],
                        start=(kt == 0), stop=False)
                nc.tensor.matmul(ps[:], lhsT=ones_r[:, 0:128],
                                 rhs=bdr[0:1, c * 512:c * 512 + 512],
                                 start=False, stop=True)
                sb = stg.tile([128, 512], BF16, tag="sb")
                nc.scalar.activation(sb[:], ps[:], AF.Identity, scale=WS)
                nc.sync.dma_start(xwd_dram[mt * 128:(mt + 1) * 128, c * 512:c * 512 + 512], sb[:])

    # =====================================================================
    # Phase A: encoder BiLSTM (fwd+bwd, independent interleaved chains)
    # =====================================================================
    with ExitStack() as phAB:
        pa = phAB.enter_context(tc.tile_pool(name="phA", bufs=1))
        Whf_q = pa.tile([128, 4 * G], FP8)
        Whb_q = pa.tile([128, 4 * G], FP8)
        nc.sync.dma_start(Whf_q[:], ins["Whhf_q"][:])
        nc.sync.dma_start(Whb_q[:], ins["Whhb_q"][:])
        encT = pa.tile([128, 8 * CBC], BF16)   # enc_out^T: cols (dt, b, l); dt<4 fwd
        hTf = pa.tile([128, 4 * 32], FP8)
        hTb = pa.tile([128, 4 * 32], FP8)
        cf = pa.tile([BL, H], F32)
        cb = pa.tile([BL, H], F32)
        tg4f = pa.tile([BL, G], F32)
        tg4b = pa.tile([BL, G], F32)
        hbf_f = pa.tile([BL, H], BF16)
        hbf_b = pa.tile([BL, H], BF16)
        for t_ in (hTf, hTb, cf, cb):
            nc.vector.memset(t_[:], 0.0)

        phAa = phAB.enter_context(ExitStack())
        xwp = phAa.enter_context(tc.tile_pool(name="xwE", bufs=4))
        eg = phAa.enter_context(tc.tile_pool(name="egp", bufs=3, space="PSUM"))
        tpE = phAa.enter_context(tc.tile_pool(name="tpE", bufs=1, space="PSUM"))

        def enc_step(t_idx, xw_src, Whh_q, hT, c_st, tg4, hbf, dvi, dt0):
            xw = xwp.tile([BL, G], BF16, tag=f"xw{dvi}")
            nc.sync.dma_start(xw[:], xw_src)
            for c in range(4):
                ps = eg.tile([32, 512], F32, tag=f"g{dvi}")
                nc.tensor.matmul(ps[:], lhsT=ident_bf[0:BL, 0:32],
                                 rhs=xw[:, c * 512:(c + 1) * 512], start=True, stop=False)
                for p in range(2):
                    nc.tensor.matmul(ps[:], lhsT=ap3(hT, (2 * p) * 32, 32, 2, 1, 32),
                                     rhs=ap3(Whh_q, (2 * p) * G + c * 512, G, 2, 1, 512),
                                     start=False, stop=(p == 1), perf_mode=DR)
                sc = (1.0 / WS) if c == 2 else (0.5 / WS)
                nc.scalar.activation(tg4[:, c * 512:(c + 1) * 512], ps[0:BL, :], AF.Tanh, scale=sc)
            ti = tg4[:, 0:512]
            tf = tg4[:, 512:1024]
            tg = tg4[:, 1024:1536]
            to = tg4[:, 1536:2048]
            nc.vector.tensor_scalar(out=ti, in0=ti, scalar1=0.5, scalar2=0.5, op0=ALU.mult, op1=ALU.add)
            nc.vector.tensor_scalar(out=tf, in0=tf, scalar1=0.5, scalar2=0.5, op0=ALU.mult, op1=ALU.add)
            nc.vector.tensor_scalar(out=to, in0=to, scalar1=0.5, scalar2=0.5, op0=ALU.mult, op1=ALU.add)
            nc.vector.tensor_tensor(out=tf, in0=tf, in1=c_st[:], op=ALU.mult)
            nc.vector.tensor_tensor(out=tg, in0=ti, in1=tg, op=ALU.mult)
            nc.vector.tensor_tensor(out=c_st[:], in0=tf, in1=tg, op=ALU.add)
            nc.scalar.activation(ti, c_st[:], AF.Tanh)
            nc.vector.tensor_tensor(out=hbf[:], in0=to, in1=ti, op=ALU.mult)
            pst = tpE.tile([128, 8], BF16, tag=f"t{dvi}")
            for j in range(4):
                nc.tensor.transpose(pst[:, 2 * j:2 * j + 2], hbf[:, j * 128:(j + 1) * 128],
                                    ident_bf[0:BL, 0:BL])
            nc.vector.tensor_copy(ap3(hT, 0, 32, 4, 1, 2), ap3(pst, 0, 2, 4, 1, 2))  # cast -> fp8
            # scatter into encT cols {(dt0+j)*CBC + b*128 + t_idx}
            dst = bass.AP(tensor=encT.tensor, offset=encT.offset + dt0 * CBC + t_idx,
                          ap=[encT.ap[0], [CBC, 4], [128, 2]])
            nc.vector.tensor_copy(dst, ap3(pst, 0, 2, 4, 1, 2))

        for t in range(LS):
            enc_step(t, xwf_dram[2 * t:2 * t + 2, :], Whf_q, hTf, cf, tg4f, hbf_f, 0, 0)
            tb = LS - 1 - t
            enc_step(tb, xwb_dram[2 * tb:2 * tb + 2, :], Whb_q, hTb, cb, tg4b, hbf_b, 1, 4)

        # decoder initial state h = [h_f; h_b], c = [c_f; c_b]
        nc.vector.tensor_copy(hT_d[:, 0:128], hTf[:])
        nc.vector.tensor_copy(hT_d[:, 128:256], hTb[:])
        nc.vector.tensor_copy(c_d[:, 0:H], cf[:])
        nc.vector.tensor_copy(c_d[:, H:H2], cb[:])
        phAa.close()

        # =================================================================
        # Phase B: preT = (enc_out @ Wbot + b_attn)^T ; encrow = enc_out rows
        # =================================================================
        with ExitStack() as phB:
            pb = phB.enter_context(tc.tile_pool(name="phB", bufs=1))
            Wbot = pb.tile([128, 8 * H2], BF16)
            nc.sync.dma_start(Wbot[:], ins["Wbot_t"][:])
            pbp = phB.enter_context(tc.tile_pool(name="phBp", bufs=2, space="PSUM"))
            for m in range(8):
                ps = pbp.tile([128, CBC], F32, tag="pre")
                for kt in range(8):
                    nc.tensor.matmul(ps[:], lhsT=Wbot[:, kt * H2 + m * 128:kt * H2 + m * 128 + 128],
                                     rhs=encT[:, kt * CBC:(kt + 1) * CBC],
                                     start=(kt == 0), stop=(kt == 7))
                nc.scalar.activation(preT_sb[:, m * CBC:(m + 1) * CBC], ps[:],
                                     AF.Identity, bias=battnT[:, m:m + 1])
            for dt in range(8):
                for b in range(BL):
                    pt2 = pbp.tile([128, 128], BF16, tag="er")
                    nc.tensor.transpose(pt2[:], encT[:, dt * CBC + b * 128:dt * CBC + b * 128 + 128],
                                        ident_bf[:, :])
                    nc.vector.tensor_copy(encrow[:, b * H2 + dt * 128:b * H2 + dt * 128 + 128], pt2[:])

    # =====================================================================
    # Phase C: attention decoder (everything SBUF-resident, fp8 DR matmuls)
    # =====================================================================
    with ExitStack() as phC:
        pc = phC.enter_context(tc.tile_pool(name="phC", bufs=1))
        hw_sb = pc.tile([BL, H2], BF16)
        hWT = pc.tile([128, 16], BF16)       # cols (dt, b)
        targ = pc.tile([128, 8 * CBC], BF16)  # tanh arg / tanh out staging
        tanh_q = pc.tile([128, 8 * CBC], FP8)
        e_sb = pc.tile([1, CBC], F32)
        wT_bf = pc.tile([128, BL], BF16)     # exp(e)^T (unnormalized)
        rZ = pc.tile([1, BL], F32)
        rZ_bf = pc.tile([1, BL], BF16)
        rep_sb = pc.tile([128, BL], F32)
        cvT_q = pc.tile([128, 8 * 32], FP8)  # cols (dt, b) b-padded to 32
        nc.vector.memset(cvT_q[:], 0.0)
        tg4 = pc.tile([BL, GD], F32)
        garg = pc.tile([BL, GD], BF16)
        h_bf = pc.tile([BL, H2], BF16)
        xw_cur = [None]

        xwp2 = phC.enter_context(tc.tile_pool(name="xwD", bufs=4))
        pg = phC.enter_context(tc.tile_pool(name="pg", bufs=1, space="PSUM"))
        phps = phC.enter_context(tc.tile_pool(name="phps", bufs=1, space="PSUM"))
        psc = phC.enter_context(tc.tile_pool(name="psc", bufs=1, space="PSUM"))
        pscb = phC.enter_context(tc.tile_pool(name="pscb", bufs=1, space="PSUM"))

        WAVE = ((0, 1, 2, 3), (4, 5, 6, 7))

        def gate_act(c, ps):
            sc = (1.0 / WS) if (c % 4) == 2 else (0.5 / WS)
            nc.scalar.activation(tg4[:, c * 512:(c + 1) * 512], ps[0:BL, :],
                                 AF.Tanh, scale=sc)

        def cell_half(hh):
            s = hh * 2048
            ti = tg4[:, s + 0:s + 512]
            tf = tg4[:, s + 512:s + 1024]
            tg = tg4[:, s + 1024:s + 1536]
            to = tg4[:, s + 1536:s + 2048]
            cs = c_d[:, hh * 512:hh * 512 + 512]
            nc.vector.tensor_scalar(out=ti, in0=ti, scalar1=0.5, scalar2=0.5, op0=ALU.mult, op1=ALU.add)
            nc.vector.tensor_scalar(out=tf, in0=tf, scalar1=0.5, scalar2=0.5, op0=ALU.mult, op1=ALU.add)
            nc.vector.tensor_scalar(out=to, in0=to, scalar1=0.5, scalar2=0.5, op0=ALU.mult, op1=ALU.add)
            nc.vector.tensor_tensor(out=tf, in0=tf, in1=cs, op=ALU.mult)
            nc.vector.tensor_tensor(out=tg, in0=ti, in1=tg, op=ALU.mult)
            nc.vector.tensor_tensor(out=cs, in0=tf, in1=tg, op=ALU.add)
            nc.scalar.activation(ti, cs, AF.Tanh)
            nc.vector.tensor_tensor(out=h_bf[:, hh * 512:hh * 512 + 512], in0=to, in1=ti,
                                    op=ALU.mult)

        for t in range(LT):
            xw = xwp2.tile([BL, GD], BF16, tag="xw")
            nc.sync.dma_start(xw[:], xwd_dram[BL * t:BL * t + BL, :])
            xw_cur[0] = xw
            scr = psc.tile([128, 512], F32, tag="sc")    # shared small-psum bank
            scb = pscb.tile([128, 32], BF16, tag="sb")   # shared bf16 psum bank

            # hW chunk 0
            psh = phps.tile([32, 512], F32, tag="hw")
            for q in range(4):
                nc.tensor.matmul(psh[:], lhsT=ap3(hT_d, (2 * q) * 32, 32, 2, 1, 32),
                                 rhs=ap3(Wtop_q, (2 * q) * H2, H2, 2, 1, 512),
                                 start=(q == 0), stop=(q == 3), perf_mode=DR)
            nc.scalar.activation(hw_sb[:, 0:512], psh[0:BL, :], AF.Identity, scale=1.0 / WS)

            psg = {}

            # hW chunk 1, then transpose -> hWT
            psh = phps.tile([32, 512], F32, tag="hw")
            for q in range(4):
                nc.tensor.matmul(psh[:], lhsT=ap3(hT_d, (2 * q) * 32, 32, 2, 1, 32),
                                 rhs=ap3(Wtop_q, (2 * q) * H2 + 512, H2, 2, 1, 512),
                                 start=(q == 0), stop=(q == 3), perf_mode=DR)
            nc.scalar.activation(hw_sb[:, 512:1024], psh[0:BL, :], AF.Identity, scale=1.0 / WS)
            pst_hw = scb[:, 0:16]
            for j in range(8):
                nc.tensor.transpose(pst_hw[:, 2 * j:2 * j + 2], hw_sb[:, j * 128:(j + 1) * 128],
                                    ident_bf[0:BL, 0:BL])
            nc.vector.tensor_copy(hWT[:], pst_hw[:])

            # wave-A: xw inject + h-part of gates (kt 8-15 of Wd)
            for c in WAVE[0]:
                ps = pg.tile([32, 512], F32, tag=f"g{c % 4}")
                psg[c] = ps
                nc.tensor.matmul(ps[:], lhsT=ident_bf[0:BL, 0:32],
                                 rhs=xw[:, c * 512:(c + 1) * 512], start=True, stop=False)
            for c in WAVE[0]:
                for q in range(4):
                    nc.tensor.matmul(psg[c][:], lhsT=ap3(hT_d, (2 * q) * 32, 32, 2, 1, 32),
                                     rhs=ap3(Wd_q, (8 + 2 * q) * GD + c * 512, GD, 2, 1, 512),
                                     start=False, stop=False, perf_mode=DR)

            # attention: targ = preT + hW (bcast over l); tanh -> fp8
            for dt in range(8):
                sl = slice(dt * CBC, (dt + 1) * CBC)
                pre3 = targ[:, sl].rearrange("p (b l) -> p b l", l=LS)
                src3 = preT_sb[:, sl].rearrange("p (b l) -> p b l", l=LS)
                hb = bass.AP(tensor=hWT.tensor, offset=hWT.offset + dt * 2,
                             ap=[hWT.ap[0], [1, BL], [0, LS]])
                nc.vector.tensor_tensor(out=pre3, in0=src3, in1=hb, op=ALU.add)
                nc.scalar.activation(tanh_q[:, sl], targ[:, sl], AF.Tanh)

            # e = v^T tanh (fp8 DR over dt pairs); psum holds 16*e (v is x16)
            pe = scr[0:32, 0:CBC]
            for p in range(4):
                nc.tensor.matmul(pe, lhsT=ap3(vT_q, (2 * p) * 32, 32, 2, 1, 32),
                                 rhs=ap3(tanh_q, (2 * p) * CBC, CBC, 2, 1, CBC),
                                 start=(p == 0), stop=(p == 3), perf_mode=DR)
            nc.scalar.activation(e_sb[:], scr[0:1, 0:CBC], AF.Identity)

            # softmax pieces: eT, exp(e) = exp(e_sb / 16)
            pet = scr[:, CBC:CBC + BL]
            for b in range(BL):
                nc.tensor.transpose(pet[:, b:b + 1], e_sb[0:1, b * LS:(b + 1) * LS], ident_f1[:])
            nc.scalar.activation(wT_bf[:], pet, AF.Exp, scale=1.0 / WS)
            pz = scr[0:1, CBC + BL:CBC + 2 * BL]
            nc.tensor.matmul(pz, lhsT=onesK[:, :], rhs=wT_bf[:], start=True, stop=True)
            nc.vector.reciprocal(rZ[:], pz)
            nc.vector.tensor_copy(rZ_bf[:], rZ[:])
            prep = scr[:, CBC + 2 * BL:CBC + 3 * BL]
            nc.tensor.matmul(prep, lhsT=ones_r[:, :], rhs=rZ_bf[:], start=True, stop=True)
            nc.vector.tensor_copy(rep_sb[:], prep)

            # cvec^T (unnormalized) then normalize+cast fp8
            pcv = scr[:, CBC + 4 * BL:CBC + 4 * BL + 16]
            for dt in range(8):
                for b in range(BL):
                    nc.tensor.matmul(pcv[:, dt * 2 + b:dt * 2 + b + 1],
                                     lhsT=encrow[:, b * H2 + dt * 128:b * H2 + dt * 128 + 128],
                                     rhs=wT_bf[:, b:b + 1], start=True, stop=True)
            rzb = bass.AP(tensor=rep_sb.tensor, offset=rep_sb.offset,
                          ap=[rep_sb.ap[0], [0, 8], [1, BL]])
            nc.vector.tensor_tensor(out=ap3(cvT_q, 0, 32, 8, 1, 2), in0=pcv, in1=rzb, op=ALU.mult)

            # wave-A cv-part (kt 0-7), close groups, acts, cell half 0
            for c in WAVE[0]:
                for q in range(4):
                    nc.tensor.matmul(psg[c][:], lhsT=ap3(cvT_q, (2 * q) * 32, 32, 2, 1, 32),
                                     rhs=ap3(Wd_q, (2 * q) * GD + c * 512, GD, 2, 1, 512),
                                     start=False, stop=(q == 3), perf_mode=DR)
                gate_act(c, psg[c])

            # wave B: inject + h-part + cv-part, acts
            for c in WAVE[1]:
                ps = pg.tile([32, 512], F32, tag=f"g{c % 4}")
                nc.tensor.matmul(ps[:], lhsT=ident_bf[0:BL, 0:32],
                                 rhs=xw[:, c * 512:(c + 1) * 512], start=True, stop=False)
                for q in range(4):
                    nc.tensor.matmul(ps[:], lhsT=ap3(hT_d, (2 * q) * 32, 32, 2, 1, 32),
                                     rhs=ap3(Wd_q, (8 + 2 * q) * GD + c * 512, GD, 2, 1, 512),
                                     start=False, stop=False, perf_mode=DR)
                for q in range(4):
                    nc.tensor.matmul(ps[:], lhsT=ap3(cvT_q, (2 * q) * 32, 32, 2, 1, 32),
                                     rhs=ap3(Wd_q, (2 * q) * GD + c * 512, GD, 2, 1, 512),
                                     start=False, stop=(q == 3), perf_mode=DR)
                gate_act(c, ps)

            cell_half(0)
            pst_h = scb[:, 16:32]
            for j in range(4):
                nc.tensor.transpose(pst_h[:, 2 * j:2 * j + 2], h_bf[:, j * 128:(j + 1) * 128],
                                    ident_bf[0:BL, 0:BL])
            cell_half(1)
            for j in range(4, 8):
                nc.tensor.transpose(pst_h[:, 2 * j:2 * j + 2], h_bf[:, j * 128:(j + 1) * 128],
                                    ident_bf[0:BL, 0:BL])
            nc.vector.tensor_copy(ap3(hT_d, 0, 32, 8, 1, 2), pst_h)
            dst = bass.AP(tensor=hsT_q.tensor, offset=hsT_q.offset + BL * t,
                          ap=[hsT_q.ap[0], [RS, 8], [1, BL]])
            nc.vector.tensor_copy(dst, pst_h)

    decw_stack.close()

    # =====================================================================
    # Phase D: full-vocab projection + log_softmax for the 2 local sequences
    # =====================================================================
    with ExitStack() as phD:
        pd = phD.enter_context(tc.tile_pool(name="phD", bufs=1))
        lg0 = pd.tile([128, V], BF16)
        lg1 = pd.tile([128, V], BF16)
        lgs = (lg0, lg1)
        zs = pd.tile([128, 2 * NVCH], F32)
        nlnZ = pd.tile([128, 2], F32)
        wotp = phD.enter_context(tc.tile_pool(name="wot", bufs=3))
        scrp = phD.enter_context(tc.tile_pool(name="scr", bufs=3))
        pdp = phD.enter_context(tc.tile_pool(name="pdp", bufs=4, space="PSUM"))

        for v in range(NVCH):
            wot = wotp.tile([128, 8 * VCH], FP8, tag="w")
            nc.sync.dma_start(wot[:], ins["WoT_q"][:, v * 8 * VCH:(v + 1) * 8 * VCH])
            bot = wotp.tile([1, VCH], BF16, tag="b")
            nc.sync.dma_start(bot[:], ins["bo"][:, v * VCH:(v + 1) * VCH])
            for mt in range(2):
                ps = pdp.tile([128, VCH], F32, tag="lg")
                for p in range(4):
                    nc.tensor.matmul(ps[:], lhsT=ap3(hsT_q, (2 * p) * RS + mt * 128, RS, 2, 1, 128),
                                     rhs=ap3(wot, (2 * p) * VCH, VCH, 2, 1, VCH),
                                     start=(p == 0), stop=False, perf_mode=DR)
                nc.tensor.matmul(ps[:], lhsT=ones_r[:, 0:128],
                                 rhs=bot[:], start=False, stop=True)
                # relu (x16-scaled psum -> /16) into the logits stash
                lsl = lgs[mt][:, v * VCH:(v + 1) * VCH]
                nc.scalar.activation(lsl, ps[:], AF.Relu, scale=1.0 / WS)
                scr = scrp.tile([128, VCH], BF16, tag="s")
                nc.scalar.activation(scr[:], lsl, AF.Exp,
                                     accum_out=zs[:, mt * NVCH + v:mt * NVCH + v + 1])
        zscr = pd.tile([128, NVCH], F32)
        ztot = pd.tile([128, 2], F32)
        for mt in range(2):
            nc.scalar.activation(zscr[:], zs[:, mt * NVCH:(mt + 1) * NVCH], AF.Identity,
                                 accum_out=ztot[:, mt:mt + 1])
        nc.scalar.activation(nlnZ[:], ztot[:], AF.Ln)
        nc.vector.tensor_scalar(out=nlnZ[:], in0=nlnZ[:], scalar1=-1.0, scalar2=None, op0=ALU.mult)

        ofp = phD.enter_context(tc.tile_pool(name="ofp", bufs=3))
        OCH = 2000
        for mt in range(2):
            for v2 in range(V // OCH):
                of = ofp.tile([128, OCH], F32, tag="of")
                src = lgs[mt][:, v2 * OCH:(v2 + 1) * OCH]
                if v2 % 2 == 0:
                    nc.scalar.activation(of[:], src, AF.Identity, bias=nlnZ[:, mt:mt + 1])
                else:
                    nc.vector.tensor_scalar(out=of[:], in0=src, scalar1=nlnZ[:, mt:mt + 1],
                                            scalar2=None, op0=ALU.add)
                nc.sync.dma_start(outs["out_shard"][mt * 128:(mt + 1) * 128, v2 * OCH:(v2 + 1) * OCH],
                                  of[:])


# ---------------------------------------------------------------------------
# host side
# ---------------------------------------------------------------------------

def _tile_k(mat: np.ndarray) -> np.ndarray:
    """[K, N] -> [128, (K//128)*N], k-tile kt at cols [kt*N,(kt+1)*N)."""
    k, n = mat.shape
    assert k % 128 == 0
    return np.ascontiguousarray(mat.reshape(k // 128, 128, n).transpose(1, 0, 2).reshape(128, -1))


def _bf(x):
    return np.asarray(x, dtype=np.float32).astype(ml_dtypes.bfloat16)


def _q8(x):
    return np.asarray(np.asarray(x, dtype=np.float32) * WS).astype(ml_dtypes.float8_e4m3fn)


def _pad32(cols):
    """[128, n] -> [128, n*32] with col j at position 32*j, zeros elsewhere."""
    out = np.zeros((128, cols.shape[1] * 32), np.float32)
    out[:, ::32] = cols
    return out


_PROG_CACHE = {}


def _build_program(n_cores):
    key = n_cores
    if key in _PROG_CACHE:
        return _PROG_CACHE[key]
    nc = bacc.Bacc("TRN2", target_bir_lowering=False, debug=False,
                   enable_asserts=False, num_devices=n_cores)
    ins = {}

    def inp(name, shape, dt):
        ins[name] = nc.dram_tensor(name, list(shape), dt, kind="ExternalInput").ap()

    inp("xsT_t", (128, 4 * CBC), BF16)
    inp("decT_t", (128, 4 * RS), BF16)
    inp("Wihf_t", (128, 4 * G), BF16)
    inp("Wihb_t", (128, 4 * G), BF16)
    inp("bfb", (1, 2 * G), BF16)
    inp("Whhf_q", (128, 4 * G), FP8)
    inp("Whhb_q", (128, 4 * G), FP8)
    inp("Wdx_t", (128, 4 * GD), BF16)
    inp("bd", (1, GD), BF16)
    inp("Wd_q", (128, 16 * GD), FP8)
    inp("Wtop_q", (128, 8 * H2), FP8)
    inp("Wbot_t", (128, 8 * H2), BF16)
    inp("battnT", (128, 8), F32)
    inp("vT_q", (128, 8 * 32), FP8)
    inp("WoT_q", (128, NVCH * 8 * VCH), FP8)
    inp("bo", (1, V), BF16)
    outs = {"out_shard": nc.dram_tensor("out_shard", [RS, V], F32,
                                        kind="ExternalOutput").ap()}
    with tile.TileContext(nc) as tc:
        _kernel_body(tc, outs, ins)
    nc.compile()
    _PROG_CACHE[key] = nc
    return nc


def prep_in_maps(inputs: dict):
    f32 = lambda k: np.asarray(inputs[k], dtype=np.float32)
    inp_idx = np.asarray(inputs["inp"]).astype(np.int64)
    tar_idx = np.asarray(inputs["tar"]).astype(np.int64)
    enc_emb = f32("enc_emb")
    dec_emb = f32("dec_emb")
    Wih_f, Whh_f, b_f = f32("Wih_f"), f32("Whh_f"), f32("b_f")
    Wih_b, Whh_b, b_b = f32("Wih_b"), f32("Whh_b"), f32("b_b")
    W_attn, b_attn, v_attn = f32("W_attn"), f32("b_attn"), f32("v_attn")
    Wih_d, Whh_d, b_d = f32("Wih_d"), f32("Whh_d"), f32("b_d")
    W_out, b_out = f32("W_out"), f32("b_out")

    # decoder gate-column permutation: chunk c (512 cols) = gate (c%4), half (c//4)
    # so PSUM wave A (c 0-3) covers i,f,g,o of hidden half 0.
    dperm = np.concatenate([np.arange((c % 4) * H2 + (c // 4) * 512,
                                      (c % 4) * H2 + (c // 4) * 512 + 512)
                            for c in range(8)])
    Wd = np.concatenate([Wih_d[E:E + H2], Whh_d], 0)[:, dperm]
    wo = _tile_k(W_out)                       # [128, 8*V], kt-major
    wo = wo.reshape(128, 8, NVCH, VCH).transpose(0, 2, 1, 3).reshape(128, -1)

    shared = {
        "Wihf_t": _bf(_tile_k(Wih_f)),
        "Wihb_t": _bf(_tile_k(Wih_b)),
        "bfb": _bf(np.concatenate([b_f, b_b]).reshape(1, 2 * G)),
        "Whhf_q": _q8(_tile_k(Whh_f)),
        "Whhb_q": _q8(_tile_k(Whh_b)),
        "Wdx_t": _bf(_tile_k(Wih_d[:E][:, dperm])),
        "bd": _bf(b_d[dperm].reshape(1, GD)),
        "Wd_q": _q8(_tile_k(Wd)),
        "Wtop_q": _q8(_tile_k(W_attn[:H2])),
        "Wbot_t": _bf(_tile_k(W_attn[H2:])),
        "battnT": np.ascontiguousarray(b_attn.reshape(8, 128).T),
        "vT_q": _q8(_pad32(v_attn.reshape(8, 128).T)),
        "WoT_q": (wo * WS).astype(ml_dtypes.float8_e4m3fn),
        "bo": _bf(WS * b_out.reshape(1, V)),
    }

    in_maps = []
    for c in range(NCORES):
        bsel = slice(BL * c, BL * (c + 1))
        xs = enc_emb[inp_idx[bsel]]                     # [2, LS, E]
        xsT = xs.reshape(CBC, E).T                      # cols (b, l)
        dec = dec_emb[tar_idx[bsel]]                    # [2, LT, E]
        decT = dec.transpose(1, 0, 2).reshape(RS, E).T  # cols (t, b)
        m = dict(shared)
        m["xsT_t"] = _bf(_tile_k(xsT))
        m["decT_t"] = _bf(_tile_k(decT))
        in_maps.append(m)
    return in_maps


LAST_EXEC_NS = None
LAST_RESULT = None


def kernel(**inputs) -> np.ndarray:
    global LAST_EXEC_NS, LAST_RESULT
    nc = _build_program(NCORES)
    in_maps = prep_in_maps(inputs)
    from concourse.bass_utils import run_bass_kernel_spmd
    trace = os.environ.get("KERNEL_TRACE") == "1"
    res = run_bass_kernel_spmd(nc, in_maps, core_ids=list(range(NCORES)),
                               trace=trace,
                               tmpdir=os.environ.get("KERNEL_TRACE_DIR"))
    LAST_EXEC_NS = res.exec_time_ns
    LAST_RESULT = res
    shards = [res.results[i]["out_shard"].reshape(LT, BL, V).transpose(1, 0, 2)
              for i in range(NCORES)]
    return np.concatenate(shards, axis=0).astype(np.float32)


# revision 4
# speedup vs baseline: 1.0995x; 1.0767x over previous
"""BiLSTM seq2seq + concat-attention + 32k-vocab log_softmax on 8 TRN2 cores.

v2 strategy (vs v1 replicate-recurrence / vocab-shard):
  - Data-parallel over batch: each core owns 2 of the 16 sequences and runs the
    FULL model for them, including the full-vocab output projection and
    log_softmax. No collectives at all; host concatenates batch shards.
  - All recurrent weight-streaming matmuls (encoder Whh, decoder Wd/Wtop, the
    attention e-reduction, and the output projection) run in fp8e4m3 with the
    DoubleRow perf mode (2 k-tiles per pass -> 2x PE throughput). Weights are
    pre-scaled x16 on the host to clear the fp8 denormal region; the 1/16 is
    folded into activation scales.
  - x @ Wih (+bias) terms for both encoder dirs and the decoder are hoisted out
    of the recurrences into one batched precompute (P0), streamed back per step
    as [2, 4H] rows and injected into PSUM via an identity matmul.
  - The whole decoder working set (Wd, Wtop, preT, enc rows, hs^T) lives in
    SBUF; the only DMA in the recurrent loops is the tiny per-step xw row.
"""
import os
import sys

sys.path.insert(0, "/opt/trn_rl_repo")

import numpy as np
import ml_dtypes
from contextlib import ExitStack

import concourse.bass as bass
import concourse.tile as tile
from concourse import bacc, mybir
from concourse._compat import with_exitstack
from concourse.masks import make_identity

BF16 = mybir.dt.bfloat16
F32 = mybir.dt.float32
FP8 = mybir.dt.float8e4
AF = mybir.ActivationFunctionType
ALU = mybir.AluOpType
DR = mybir.MatmulPerfMode.DoubleRow

# problem constants (kernel.py must be self-contained)
B, E, H, H2 = 16, 512, 512, 1024
G = 4 * H           # 2048 encoder gate width
GD = 4 * H2         # 4096 decoder gate width
V = 32000
LS = LT = 128
NCORES = 8
BL = B // NCORES    # 2 sequences per core
RS = BL * LT        # 256 decoder rows, (t, b)-major
CBC = BL * LS       # 256 attention columns per core, (b, l)-major
WS = 16.0           # fp8 weight pre-scale
VCH = 500           # phase-D vocab chunk (one PSUM bank)
NVCH = V // VCH     # 64


def ap3(t, off, s1, c1, s2, c2):
    """3-dim AP view of tile t: [partition, [s1,c1], [s2,c2]] at element offset."""
    return bass.AP(tensor=t.tensor, offset=t.offset + off,
                   ap=[t.ap[0], [s1, c1], [s2, c2]])


@with_exitstack
def _kernel_body(ctx: ExitStack, tc: tile.TileContext, outs, ins):
    nc = tc.nc

    dram = ctx.enter_context(tc.tile_pool(name="dram", bufs=1, space="DRAM"))
    const = ctx.enter_context(tc.tile_pool(name="const", bufs=1))

    ident_bf = const.tile([128, 128], BF16)
    make_identity(nc, ident_bf[:])
    ident_f1 = const.tile([1, 1], F32)
    nc.vector.memset(ident_f1[:], 1.0)
    ones_r = const.tile([1, 128], BF16)     # row of ones (bias matmuls / bcast)
    nc.vector.memset(ones_r[:], 1.0)
    onesK = const.tile([128, 1], BF16)      # column of ones (Z sums)
    nc.vector.memset(onesK[:], 1.0)
    vT_q = const.tile([128, 8 * 32], FP8)
    nc.sync.dma_start(vT_q[:], ins["vT_q"][:])
    battnT = const.tile([128, 8], F32)
    nc.sync.dma_start(battnT[:], ins["battnT"][:])

    # DRAM intermediates: hoisted x-projections
    xwf_dram = dram.tile([CBC, G], BF16)    # rows (l, b): row = 2*l + b
    xwb_dram = dram.tile([CBC, G], BF16)
    xwd_dram = dram.tile([RS, GD], BF16)    # rows (t, b): row = 2*t + b

    # decoder-persistent SBUF (allocated up front; loaded while P0/enc run).
    # decw closes before phase D to make room for the logits stash.
    decw_stack = ctx.enter_context(ExitStack())
    dec = decw_stack.enter_context(tc.tile_pool(name="dec", bufs=1))
    Wd_q = dec.tile([128, 16 * GD], FP8)    # kt 0-7 cv-part, 8-15 h-part (x16)
    nc.sync.dma_start(Wd_q[:], ins["Wd_q"][:])
    Wtop_q = dec.tile([128, 8 * H2], FP8)   # attention h-projection (x16)
    nc.sync.dma_start(Wtop_q[:], ins["Wtop_q"][:])
    preT_sb = dec.tile([128, 8 * CBC], BF16)   # (enc_out @ Wbot + b)^T, dt-major
    encrow = dec.tile([128, BL * H2], BF16)    # enc_out rows: [l, (b, d)]
    hsT_q = const.tile([128, 8 * RS], FP8)     # decoder hs^T k-tiles for phase D
    hT_d = const.tile([128, 8 * 32], FP8)      # decoder h^T, cols (kt, b) b-padded to 32
    c_d = const.tile([BL, H2], F32)
    nc.vector.memset(hT_d[:], 0.0)

    # =====================================================================
    # P0: xwf/xwb/xwd = x @ Wih (+bias), x16, to DRAM
    # =====================================================================
    with ExitStack() as ph0:
        p0 = ph0.enter_context(tc.tile_pool(name="p0", bufs=1))
        xsT = p0.tile([128, 4 * CBC], BF16)
        decT = p0.tile([128, 4 * RS], BF16)
        Wf = p0.tile([128, 4 * G], BF16)
        Wb = p0.tile([128, 4 * G], BF16)
        Wdx = p0.tile([128, 4 * GD], BF16)
        bfb = p0.tile([1, 2 * G], BF16)
        bdr = p0.tile([1, GD], BF16)
        nc.sync.dma_start(xsT[:], ins["xsT_t"][:])
        nc.sync.dma_start(decT[:], ins["decT_t"][:])
        nc.sync.dma_start(Wf[:], ins["Wihf_t"][:])
        nc.sync.dma_start(Wb[:], ins["Wihb_t"][:])
        nc.sync.dma_start(Wdx[:], ins["Wdx_t"][:])
        nc.sync.dma_start(bfb[:], ins["bfb"][:])
        nc.sync.dma_start(bdr[:], ins["bd"][:])
        stg = ph0.enter_context(tc.tile_pool(name="p0s", bufs=3))
        pp = ph0.enter_context(tc.tile_pool(name="p0p", bufs=3, space="PSUM"))

        for W, bi, dst in ((Wf, 0, xwf_dram), (Wb, 1, xwb_dram)):
            for mt in range(BL):       # batch-elem m-tile (xsT cols are b-major)
                for c in range(4):
                    ps = pp.tile([128, 512], F32, tag="ps")
                    for kt in range(4):
                        nc.tensor.matmul(
                            ps[:], lhsT=xsT[:, kt * CBC + mt * 128:kt * CBC + mt * 128 + 128],
                            rhs=W[:, kt * G + c * 512:kt * G + c * 512 + 512],
                            start=(kt == 0), stop=False)
                    nc.tensor.matmul(ps[:], lhsT=ones_r[:, 0:128],
                                     rhs=bfb[0:1, bi * G + c * 512:bi * G + c * 512 + 512],
                                     start=False, stop=True)
                    sb = stg.tile([128, 512], BF16, tag="sb")
                    nc.scalar.activation(sb[:], ps[:], AF.Identity, scale=WS)
                    dst_ap = bass.AP(tensor=dst.tensor,
                                     offset=dst.offset + mt * G + c * 512,
                                     ap=[[BL * G, 128], [1, 512]])
                    nc.sync.dma_start(dst_ap, sb[:])
        for mt in range(2):            # row m-tiles of (t, b)
            for c in range(8):
                ps = pp.tile([128, 512], F32, tag="ps")
                for kt in range(4):
                    nc.tensor.matmul(
                        ps[:], lhsT=decT[:, kt * RS + mt * 128:kt * RS + mt * 128 + 128],
                        rhs=Wdx[:, kt * GD + c * 512:kt * GD + c * 512 + 512],
                        start=(kt == 0), stop=False)
                nc.tensor.matmul(ps[:], lhsT=ones_r[:, 0:128],
                                 rhs=bdr[0:1, c * 512:c * 512 + 512],
                                 start=False, stop=True)
                sb = stg.tile([128, 512], BF16, tag="sb")
                nc.scalar.activation(sb[:], ps[:], AF.Identity, scale=WS)
                nc.sync.dma_start(xwd_dram[mt * 128:(mt + 1) * 128, c * 512:c * 512 + 512], sb[:])

    # =====================================================================
    # Phase A: encoder BiLSTM (fwd+bwd, independent interleaved chains)
    # =====================================================================
    with ExitStack() as phAB:
        pa = phAB.enter_context(tc.tile_pool(name="phA", bufs=1))
        Whf_q = pa.tile([128, 4 * G], FP8)
        Whb_q = pa.tile([128, 4 * G], FP8)
        nc.sync.dma_start(Whf_q[:], ins["Whhf_q"][:])
        nc.sync.dma_start(Whb_q[:], ins["Whhb_q"][:])
        encT = pa.tile([128, 8 * CBC], BF16)   # enc_out^T: cols (dt, b, l); dt<4 fwd
        hTf = pa.tile([128, 4 * 32], FP8)
        hTb = pa.tile([128, 4 * 32], FP8)
        cf = pa.tile([BL, H], F32)
        cb = pa.tile([BL, H], F32)
        tg4f = pa.tile([BL, G], F32)
        tg4b = pa.tile([BL, G], F32)
        hbf_f = pa.tile([BL, H], BF16)
        hbf_b = pa.tile([BL, H], BF16)
        for t_ in (hTf, hTb, cf, cb):
            nc.vector.memset(t_[:], 0.0)

        phAa = phAB.enter_context(ExitStack())
        xwp = phAa.enter_context(tc.tile_pool(name="xwE", bufs=4))
        eg = phAa.enter_context(tc.tile_pool(name="egp", bufs=3, space="PSUM"))
        tpE = phAa.enter_context(tc.tile_pool(name="tpE", bufs=1, space="PSUM"))

        def enc_step(t_idx, xw_src, Whh_q, hT, c_st, tg4, hbf, dvi, dt0):
            xw = xwp.tile([BL, G], BF16, tag=f"xw{dvi}")
            nc.sync.dma_start(xw[:], xw_src)
            for c in range(4):
                ps = eg.tile([32, 512], F32, tag=f"g{dvi}")
                nc.tensor.matmul(ps[:], lhsT=ident_bf[0:BL, 0:32],
                                 rhs=xw[:, c * 512:(c + 1) * 512], start=True, stop=False)
                for p in range(2):
                    nc.tensor.matmul(ps[:], lhsT=ap3(hT, (2 * p) * 32, 32, 2, 1, 32),
                                     rhs=ap3(Whh_q, (2 * p) * G + c * 512, G, 2, 1, 512),
                                     start=False, stop=(p == 1), perf_mode=DR)
                fn = AF.Tanh if c == 2 else AF.Sigmoid
                nc.scalar.activation(tg4[:, c * 512:(c + 1) * 512], ps[0:BL, :], fn,
                                     scale=1.0 / WS)
            ti = tg4[:, 0:512]
            tf = tg4[:, 512:1024]
            tg = tg4[:, 1024:1536]
            to = tg4[:, 1536:2048]
            nc.vector.tensor_tensor(out=tf, in0=tf, in1=c_st[:], op=ALU.mult)
            nc.vector.tensor_tensor(out=tg, in0=ti, in1=tg, op=ALU.mult)
            nc.vector.tensor_tensor(out=c_st[:], in0=tf, in1=tg, op=ALU.add)
            nc.scalar.activation(ti, c_st[:], AF.Tanh)
            nc.vector.tensor_tensor(out=hbf[:], in0=to, in1=ti, op=ALU.mult)
            pst = tpE.tile([128, 8], BF16, tag=f"t{dvi}")
            for j in range(4):
                nc.tensor.transpose(pst[:, 2 * j:2 * j + 2], hbf[:, j * 128:(j + 1) * 128],
                                    ident_bf[0:BL, 0:BL])
            nc.vector.tensor_copy(ap3(hT, 0, 32, 4, 1, 2), ap3(pst, 0, 2, 4, 1, 2))  # cast -> fp8
            # scatter into encT cols {(dt0+j)*CBC + b*128 + t_idx}
            dst = bass.AP(tensor=encT.tensor, offset=encT.offset + dt0 * CBC + t_idx,
                          ap=[encT.ap[0], [CBC, 4], [128, 2]])
            nc.vector.tensor_copy(dst, ap3(pst, 0, 2, 4, 1, 2))

        for t in range(LS):
            enc_step(t, xwf_dram[2 * t:2 * t + 2, :], Whf_q, hTf, cf, tg4f, hbf_f, 0, 0)
            tb = LS - 1 - t
            enc_step(tb, xwb_dram[2 * tb:2 * tb + 2, :], Whb_q, hTb, cb, tg4b, hbf_b, 1, 4)

        # decoder initial state h = [h_f; h_b], c = [c_f; c_b]
        nc.vector.tensor_copy(hT_d[:, 0:128], hTf[:])
        nc.vector.tensor_copy(hT_d[:, 128:256], hTb[:])
        nc.vector.tensor_copy(c_d[:, 0:H], cf[:])
        nc.vector.tensor_copy(c_d[:, H:H2], cb[:])
        phAa.close()

        # =================================================================
        # Phase B: preT = (enc_out @ Wbot + b_attn)^T ; encrow = enc_out rows
        # =================================================================
        with ExitStack() as phB:
            pb = phB.enter_context(tc.tile_pool(name="phB", bufs=1))
            Wbot = pb.tile([128, 8 * H2], BF16)
            nc.sync.dma_start(Wbot[:], ins["Wbot_t"][:])
            pbp = phB.enter_context(tc.tile_pool(name="phBp", bufs=2, space="PSUM"))
            for m in range(8):
                ps = pbp.tile([128, CBC], F32, tag="pre")
                for kt in range(8):
                    nc.tensor.matmul(ps[:], lhsT=Wbot[:, kt * H2 + m * 128:kt * H2 + m * 128 + 128],
                                     rhs=encT[:, kt * CBC:(kt + 1) * CBC],
                                     start=(kt == 0), stop=(kt == 7))
                nc.scalar.activation(preT_sb[:, m * CBC:(m + 1) * CBC], ps[:],
                                     AF.Identity, bias=battnT[:, m:m + 1])
            for dt in range(8):
                for b in range(BL):
                    pt2 = pbp.tile([128, 128], BF16, tag="er")
                    nc.tensor.transpose(pt2[:], encT[:, dt * CBC + b * 128:dt * CBC + b * 128 + 128],
                                        ident_bf[:, :])
                    nc.vector.tensor_copy(encrow[:, b * H2 + dt * 128:b * H2 + dt * 128 + 128], pt2[:])

    # =====================================================================
    # Phase C: attention decoder (everything SBUF-resident, fp8 DR matmuls)
    # =====================================================================
    with ExitStack() as phC:
        pc = phC.enter_context(tc.tile_pool(name="phC", bufs=1))
        hw_sb = pc.tile([BL, H2], BF16)
        hWT = pc.tile([128, 16], BF16)       # cols (dt, b)
        targ = pc.tile([128, 8 * CBC], BF16)  # tanh arg / tanh out staging
        tanh_q = pc.tile([128, 8 * CBC], FP8)
        e_sb = pc.tile([1, CBC], F32)
        wT_bf = pc.tile([128, BL], BF16)     # exp(e)^T (unnormalized)
        rZ = pc.tile([1, BL], F32)
        rZ_bf = pc.tile([1, BL], BF16)
        rep_sb = pc.tile([128, BL], F32)
        cvT_q = pc.tile([128, 8 * 32], FP8)  # cols (dt, b) b-padded to 32
        nc.vector.memset(cvT_q[:], 0.0)
        tg4 = pc.tile([BL, GD], F32)
        garg = pc.tile([BL, GD], BF16)
        h_bf = pc.tile([BL, H2], BF16)
        xw_cur = [None]

        xwp2 = phC.enter_context(tc.tile_pool(name="xwD", bufs=4))
        pg = phC.enter_context(tc.tile_pool(name="pg", bufs=1, space="PSUM"))
        phps = phC.enter_context(tc.tile_pool(name="phps", bufs=1, space="PSUM"))
        psc = phC.enter_context(tc.tile_pool(name="psc", bufs=1, space="PSUM"))
        pscb = phC.enter_context(tc.tile_pool(name="pscb", bufs=1, space="PSUM"))

        WAVE = ((0, 1, 2, 3), (4, 5, 6, 7))

        def gate_act(c, ps):
            sc = (1.0 / WS) if (c % 4) == 2 else (0.5 / WS)
            nc.scalar.activation(tg4[:, c * 512:(c + 1) * 512], ps[0:BL, :],
                                 AF.Tanh, scale=sc)

        def cell_half(hh):
            s = hh * 2048
            ti = tg4[:, s + 0:s + 512]
            tf = tg4[:, s + 512:s + 1024]
            tg = tg4[:, s + 1024:s + 1536]
            to = tg4[:, s + 1536:s + 2048]
            cs = c_d[:, hh * 512:hh * 512 + 512]
            nc.vector.tensor_scalar(out=ti, in0=ti, scalar1=0.5, scalar2=0.5, op0=ALU.mult, op1=ALU.add)
            nc.vector.tensor_scalar(out=tf, in0=tf, scalar1=0.5, scalar2=0.5, op0=ALU.mult, op1=ALU.add)
            nc.vector.tensor_scalar(out=to, in0=to, scalar1=0.5, scalar2=0.5, op0=ALU.mult, op1=ALU.add)
            nc.vector.tensor_tensor(out=tf, in0=tf, in1=cs, op=ALU.mult)
            nc.vector.tensor_tensor(out=tg, in0=ti, in1=tg, op=ALU.mult)
            nc.vector.tensor_tensor(out=cs, in0=tf, in1=tg, op=ALU.add)
            nc.scalar.activation(ti, cs, AF.Tanh)
            nc.vector.tensor_tensor(out=h_bf[:, hh * 512:hh * 512 + 512], in0=to, in1=ti,
                                    op=ALU.mult)

        for t in range(LT):
            xw = xwp2.tile([BL, GD], BF16, tag="xw")
            nc.sync.dma_start(xw[:], xwd_dram[BL * t:BL * t + BL, :])
            xw_cur[0] = xw
            scr = psc.tile([128, 512], F32, tag="sc")    # shared small-psum bank
            scb = pscb.tile([128, 32], BF16, tag="sb")   # shared bf16 psum bank

            # hW chunk 0
            psh = phps.tile([32, 512], F32, tag="hw")
            for q in range(4):
                nc.tensor.matmul(psh[:], lhsT=ap3(hT_d, (2 * q) * 32, 32, 2, 1, 32),
                                 rhs=ap3(Wtop_q, (2 * q) * H2, H2, 2, 1, 512),
                                 start=(q == 0), stop=(q == 3), perf_mode=DR)
            nc.scalar.activation(hw_sb[:, 0:512], psh[0:BL, :], AF.Identity, scale=1.0 / WS)

            psg = {}

            # hW chunk 1, then transpose -> hWT
            psh = phps.tile([32, 512], F32, tag="hw")
            for q in range(4):
                nc.tensor.matmul(psh[:], lhsT=ap3(hT_d, (2 * q) * 32, 32, 2, 1, 32),
                                 rhs=ap3(Wtop_q, (2 * q) * H2 + 512, H2, 2, 1, 512),
                                 start=(q == 0), stop=(q == 3), perf_mode=DR)
            nc.scalar.activation(hw_sb[:, 512:1024], psh[0:BL, :], AF.Identity, scale=1.0 / WS)
            pst_hw = scb[:, 0:16]
            for j in range(8):
                nc.tensor.transpose(pst_hw[:, 2 * j:2 * j + 2], hw_sb[:, j * 128:(j + 1) * 128],
                                    ident_bf[0:BL, 0:BL])
            nc.vector.tensor_copy(hWT[:], pst_hw[:])

            # wave-A: xw inject + h-part of gates (kt 8-15 of Wd)
            for c in WAVE[0]:
                ps = pg.tile([32, 512], F32, tag=f"g{c % 4}")
                psg[c] = ps
                nc.tensor.matmul(ps[:], lhsT=ident_bf[0:BL, 0:32],
                                 rhs=xw[:, c * 512:(c + 1) * 512], start=True, stop=False)
            for c in WAVE[0]:
                for q in range(4):
                    nc.tensor.matmul(psg[c][:], lhsT=ap3(hT_d, (2 * q) * 32, 32, 2, 1, 32),
                                     rhs=ap3(Wd_q, (8 + 2 * q) * GD + c * 512, GD, 2, 1, 512),
                                     start=False, stop=False, perf_mode=DR)

            # attention: targ = preT + hW (bcast over l); tanh -> fp8
            for dt in range(8):
                sl = slice(dt * CBC, (dt + 1) * CBC)
                pre3 = targ[:, sl].rearrange("p (b l) -> p b l", l=LS)
                src3 = preT_sb[:, sl].rearrange("p (b l) -> p b l", l=LS)
                hb = bass.AP(tensor=hWT.tensor, offset=hWT.offset + dt * 2,
                             ap=[hWT.ap[0], [1, BL], [0, LS]])
                nc.vector.tensor_tensor(out=pre3, in0=src3, in1=hb, op=ALU.add)
                nc.scalar.activation(tanh_q[:, sl], targ[:, sl], AF.Tanh)

            # e = v^T tanh (fp8 DR over dt pairs); psum holds 16*e (v is x16)
            pe = scr[0:32, 0:CBC]
            for p in range(4):
                nc.tensor.matmul(pe, lhsT=ap3(vT_q, (2 * p) * 32, 32, 2, 1, 32),
                                 rhs=ap3(tanh_q, (2 * p) * CBC, CBC, 2, 1, CBC),
                                 start=(p == 0), stop=(p == 3), perf_mode=DR)
            nc.scalar.activation(e_sb[:], scr[0:1, 0:CBC], AF.Identity)

            # softmax pieces: eT, exp(e) = exp(e_sb / 16)
            pet = scr[:, CBC:CBC + BL]
            for b in range(BL):
                nc.tensor.transpose(pet[:, b:b + 1], e_sb[0:1, b * LS:(b + 1) * LS], ident_f1[:])
            nc.scalar.activation(wT_bf[:], pet, AF.Exp, scale=1.0 / WS)
            pz = scr[0:1, CBC + BL:CBC + 2 * BL]
            nc.tensor.matmul(pz, lhsT=onesK[:, :], rhs=wT_bf[:], start=True, stop=True)
            nc.vector.reciprocal(rZ[:], pz)
            nc.vector.tensor_copy(rZ_bf[:], rZ[:])
            prep = scr[:, CBC + 2 * BL:CBC + 3 * BL]
            nc.tensor.matmul(prep, lhsT=ones_r[:, :], rhs=rZ_bf[:], start=True, stop=True)
            nc.vector.tensor_copy(rep_sb[:], prep)

            # cvec^T (unnormalized) then normalize+cast fp8
            pcv = scr[:, CBC + 4 * BL:CBC + 4 * BL + 16]
            for dt in range(8):
                for b in range(BL):
                    nc.tensor.matmul(pcv[:, dt * 2 + b:dt * 2 + b + 1],
                                     lhsT=encrow[:, b * H2 + dt * 128:b * H2 + dt * 128 + 128],
                                     rhs=wT_bf[:, b:b + 1], start=True, stop=True)
            rzb = bass.AP(tensor=rep_sb.tensor, offset=rep_sb.offset,
                          ap=[rep_sb.ap[0], [0, 8], [1, BL]])
            nc.vector.tensor_tensor(out=ap3(cvT_q, 0, 32, 8, 1, 2), in0=pcv, in1=rzb, op=ALU.mult)

            # wave-A cv-part (kt 0-7), close groups, acts, cell half 0
            for c in WAVE[0]:
                for q in range(4):
                    nc.tensor.matmul(psg[c][:], lhsT=ap3(cvT_q, (2 * q) * 32, 32, 2, 1, 32),
                                     rhs=ap3(Wd_q, (2 * q) * GD + c * 512, GD, 2, 1, 512),
                                     start=False, stop=(q == 3), perf_mode=DR)
                gate_act(c, psg[c])

            # wave B: inject + h-part + cv-part, acts
            for c in WAVE[1]:
                ps = pg.tile([32, 512], F32, tag=f"g{c % 4}")
                nc.tensor.matmul(ps[:], lhsT=ident_bf[0:BL, 0:32],
                                 rhs=xw[:, c * 512:(c + 1) * 512], start=True, stop=False)
                for q in range(4):
                    nc.tensor.matmul(ps[:], lhsT=ap3(hT_d, (2 * q) * 32, 32, 2, 1, 32),
                                     rhs=ap3(Wd_q, (8 + 2 * q) * GD + c * 512, GD, 2, 1, 512),
                                     start=False, stop=False, perf_mode=DR)
                for q in range(4):
                    nc.tensor.matmul(ps[:], lhsT=ap3(cvT_q, (2 * q) * 32, 32, 2, 1, 32),
                                     rhs=ap3(Wd_q, (2 * q) * GD + c * 512, GD, 2, 1, 512),
                                     start=False, stop=(q == 3), perf_mode=DR)
                gate_act(c, ps)

            cell_half(0)
            pst_h = scb[:, 16:32]
            for j in range(4):
                nc.tensor.transpose(pst_h[:, 2 * j:2 * j + 2], h_bf[:, j * 128:(j + 1) * 128],
                                    ident_bf[0:BL, 0:BL])
            cell_half(1)
            for j in range(4, 8):
                nc.tensor.transpose(pst_h[:, 2 * j:2 * j + 2], h_bf[:, j * 128:(j + 1) * 128],
                                    ident_bf[0:BL, 0:BL])
            nc.vector.tensor_copy(ap3(hT_d, 0, 32, 8, 1, 2), pst_h)
            dst = bass.AP(tensor=hsT_q.tensor, offset=hsT_q.offset + BL * t,
                          ap=[hsT_q.ap[0], [RS, 8], [1, BL]])
            nc.vector.tensor_copy(dst, pst_h)

    decw_stack.close()

    # =====================================================================
    # Phase D: full-vocab projection + log_softmax for the 2 local sequences
    # =====================================================================
    with ExitStack() as phD:
        pd = phD.enter_context(tc.tile_pool(name="phD", bufs=1))
        lg0 = pd.tile([128, V], BF16)
        lg1 = pd.tile([128, V], BF16)
        lgs = (lg0, lg1)
        zs = pd.tile([128, 2 * NVCH], F32)
        nlnZ = pd.tile([128, 2], F32)
        wotp = phD.enter_context(tc.tile_pool(name="wot", bufs=3))
        scrp = phD.enter_context(tc.tile_pool(name="scr", bufs=3))
        pdp = phD.enter_context(tc.tile_pool(name="pdp", bufs=4, space="PSUM"))

        for v in range(NVCH):
            wot = wotp.tile([128, 8 * VCH], FP8, tag="w")
            nc.sync.dma_start(wot[:], ins["WoT_q"][:, v * 8 * VCH:(v + 1) * 8 * VCH])
            bot = wotp.tile([1, VCH], BF16, tag="b")
            nc.sync.dma_start(bot[:], ins["bo"][:, v * VCH:(v + 1) * VCH])
            for mt in range(2):
                ps = pdp.tile([128, VCH], F32, tag="lg")
                for p in range(4):
                    nc.tensor.matmul(ps[:], lhsT=ap3(hsT_q, (2 * p) * RS + mt * 128, RS, 2, 1, 128),
                                     rhs=ap3(wot, (2 * p) * VCH, VCH, 2, 1, VCH),
                                     start=(p == 0), stop=False, perf_mode=DR)
                nc.tensor.matmul(ps[:], lhsT=ones_r[:, 0:128],
                                 rhs=bot[:], start=False, stop=True)
                # relu (x16-scaled psum -> /16) into the logits stash
                lsl = lgs[mt][:, v * VCH:(v + 1) * VCH]
                nc.scalar.activation(lsl, ps[:], AF.Relu, scale=1.0 / WS)
                scr = scrp.tile([128, VCH], BF16, tag="s")
                nc.scalar.activation(scr[:], lsl, AF.Exp,
                                     accum_out=zs[:, mt * NVCH + v:mt * NVCH + v + 1])
        zscr = pd.tile([128, NVCH], F32)
        ztot = pd.tile([128, 2], F32)
        for mt in range(2):
            nc.scalar.activation(zscr[:], zs[:, mt * NVCH:(mt + 1) * NVCH], AF.Identity,
                                 accum_out=ztot[:, mt:mt + 1])
        nc.scalar.activation(nlnZ[:], ztot[:], AF.Ln)
        nc.vector.tensor_scalar(out=nlnZ[:], in0=nlnZ[:], scalar1=-1.0, scalar2=None, op0=ALU.mult)

        ofp = phD.enter_context(tc.tile_pool(name="ofp", bufs=3))
        OCH = 2000
        for mt in range(2):
            for v2 in range(V // OCH):
                of = ofp.tile([128, OCH], F32, tag="of")
                src = lgs[mt][:, v2 * OCH:(v2 + 1) * OCH]
                if v2 % 2 == 0:
                    nc.scalar.activation(of[:], src, AF.Identity, bias=nlnZ[:, mt:mt + 1])
                else:
                    nc.vector.tensor_scalar(out=of[:], in0=src, scalar1=nlnZ[:, mt:mt + 1],
                                            scalar2=None, op0=ALU.add)
                nc.sync.dma_start(outs["out_shard"][mt * 128:(mt + 1) * 128, v2 * OCH:(v2 + 1) * OCH],
                                  of[:])


# ---------------------------------------------------------------------------
# host side
# ---------------------------------------------------------------------------

def _tile_k(mat: np.ndarray) -> np.ndarray:
    """[K, N] -> [128, (K//128)*N], k-tile kt at cols [kt*N,(kt+1)*N)."""
    k, n = mat.shape
    assert k % 128 == 0
    return np.ascontiguousarray(mat.reshape(k // 128, 128, n).transpose(1, 0, 2).reshape(128, -1))


def _bf(x):
    return np.asarray(x, dtype=np.float32).astype(ml_dtypes.bfloat16)


def _q8(x):
    return np.asarray(np.asarray(x, dtype=np.float32) * WS).astype(ml_dtypes.float8_e4m3fn)


def _pad32(cols):
    """[128, n] -> [128, n*32] with col j at position 32*j, zeros elsewhere."""
    out = np.zeros((128, cols.shape[1] * 32), np.float32)
    out[:, ::32] = cols
    return out


_PROG_CACHE = {}


def _build_program(n_cores):
    key = n_cores
    if key in _PROG_CACHE:
        return _PROG_CACHE[key]
    nc = bacc.Bacc("TRN2", target_bir_lowering=False, debug=False,
                   enable_asserts=False, num_devices=n_cores)
    ins = {}

    def inp(name, shape, dt):
        ins[name] = nc.dram_tensor(name, list(shape), dt, kind="ExternalInput").ap()

    inp("xsT_t", (128, 4 * CBC), BF16)
    inp("decT_t", (128, 4 * RS), BF16)
    inp("Wihf_t", (128, 4 * G), BF16)
    inp("Wihb_t", (128, 4 * G), BF16)
    inp("bfb", (1, 2 * G), BF16)
    inp("Whhf_q", (128, 4 * G), FP8)
    inp("Whhb_q", (128, 4 * G), FP8)
    inp("Wdx_t", (128, 4 * GD), BF16)
    inp("bd", (1, GD), BF16)
    inp("Wd_q", (128, 16 * GD), FP8)
    inp("Wtop_q", (128, 8 * H2), FP8)
    inp("Wbot_t", (128, 8 * H2), BF16)
    inp("battnT", (128, 8), F32)
    inp("vT_q", (128, 8 * 32), FP8)
    inp("WoT_q", (128, NVCH * 8 * VCH), FP8)
    inp("bo", (1, V), BF16)
    outs = {"out_shard": nc.dram_tensor("out_shard", [RS, V], F32,
                                        kind="ExternalOutput").ap()}
    with tile.TileContext(nc) as tc:
        _kernel_body(tc, outs, ins)
    nc.compile()
    _PROG_CACHE[key] = nc
    return nc


def prep_in_maps(inputs: dict):
    f32 = lambda k: np.asarray(inputs[k], dtype=np.float32)
    inp_idx = np.asarray(inputs["inp"]).astype(np.int64)
    tar_idx = np.asarray(inputs["tar"]).astype(np.int64)
    enc_emb = f32("enc_emb")
    dec_emb = f32("dec_emb")
    Wih_f, Whh_f, b_f = f32("Wih_f"), f32("Whh_f"), f32("b_f")
    Wih_b, Whh_b, b_b = f32("Wih_b"), f32("Whh_b"), f32("b_b")
    W_attn, b_attn, v_attn = f32("W_attn"), f32("b_attn"), f32("v_attn")
    Wih_d, Whh_d, b_d = f32("Wih_d"), f32("Whh_d"), f32("b_d")
    W_out, b_out = f32("W_out"), f32("b_out")

    # decoder gate-column permutation: chunk c (512 cols) = gate (c%4), half (c//4)
    # so PSUM wave A (c 0-3) covers i,f,g,o of hidden half 0.
    dperm = np.concatenate([np.arange((c % 4) * H2 + (c // 4) * 512,
                                      (c % 4) * H2 + (c // 4) * 512 + 512)
                            for c in range(8)])
    Wd = np.concatenate([Wih_d[E:E + H2], Whh_d], 0)[:, dperm]
    wo = _tile_k(W_out)                       # [128, 8*V], kt-major
    wo = wo.reshape(128, 8, NVCH, VCH).transpose(0, 2, 1, 3).reshape(128, -1)

    shared = {
        "Wihf_t": _bf(_tile_k(Wih_f)),
        "Wihb_t": _bf(_tile_k(Wih_b)),
        "bfb": _bf(np.concatenate([b_f, b_b]).reshape(1, 2 * G)),
        "Whhf_q": _q8(_tile_k(Whh_f)),
        "Whhb_q": _q8(_tile_k(Whh_b)),
        "Wdx_t": _bf(_tile_k(Wih_d[:E][:, dperm])),
        "bd": _bf(b_d[dperm].reshape(1, GD)),
        "Wd_q": _q8(_tile_k(Wd)),
        "Wtop_q": _q8(_tile_k(W_attn[:H2])),
        "Wbot_t": _bf(_tile_k(W_attn[H2:])),
        "battnT": np.ascontiguousarray(b_attn.reshape(8, 128).T),
        "vT_q": _q8(_pad32(v_attn.reshape(8, 128).T)),
        "WoT_q": (wo * WS).astype(ml_dtypes.float8_e4m3fn),
        "bo": _bf(WS * b_out.reshape(1, V)),
    }

    in_maps = []
    for c in range(NCORES):
        bsel = slice(BL * c, BL * (c + 1))
        xs = enc_emb[inp_idx[bsel]]                     # [2, LS, E]
        xsT = xs.reshape(CBC, E).T                      # cols (b, l)
        dec = dec_emb[tar_idx[bsel]]                    # [2, LT, E]
        decT = dec.transpose(1, 0, 2).reshape(RS, E).T  # cols (t, b)
        m = dict(shared)
        m["xsT_t"] = _bf(_tile_k(xsT))
        m["decT_t"] = _bf(_tile_k(decT))
        in_maps.append(m)
    return in_maps


LAST_EXEC_NS = None
LAST_RESULT = None


def kernel(**inputs) -> np.ndarray:
    global LAST_EXEC_NS, LAST_RESULT
    nc = _build_program(NCORES)
    in_maps = prep_in_maps(inputs)
    from concourse.bass_utils import run_bass_kernel_spmd
    trace = os.environ.get("KERNEL_TRACE") == "1"
    res = run_bass_kernel_spmd(nc, in_maps, core_ids=list(range(NCORES)),
                               trace=trace,
                               tmpdir=os.environ.get("KERNEL_TRACE_DIR"))
    LAST_EXEC_NS = res.exec_time_ns
    LAST_RESULT = res
    shards = [res.results[i]["out_shard"].reshape(LT, BL, V).transpose(1, 0, 2)
              for i in range(NCORES)]
    return np.concatenate(shards, axis=0).astype(np.float32)
